# revision 16
# baseline (speedup 1.0000x reference)
"""TRN2 Bass kernel for nn_CMDB: cross-modal Mamba(S6) fusion block.

Sharding: 8 cores; core c handles blk = c//4 (0: d2r, 1: r2d), sample
b = c%4. Each core runs the full cmfb+CBAM for its (blk, b) half, then the
pair exchanges halves (AllGather) and both redundantly compute the final
FFN for sample b; host takes cores 0-3 outputs.

Self-contained: hardcodes all shapes; host does only slicing/layout prep of
weights (data-independent) and shard assembly.
"""
import sys
import numpy as np

sys.path.insert(0, "/root/problem")
sys.path.insert(0, "/opt/trn_rl_repo")

import concourse.bass as bass
import concourse.mybir as mybir
import concourse.tile as tile
from concourse import bass_utils

# ---- inlined walrus single-wait workaround (see tile_fixups.py) ----


import bass_rust

MAX_WAITS = 1

_installed = False
_orig_add_instruction = tile.TileContext._add_instruction


def _patched_add_instruction(self, inst):
    si = inst.sync_info
    if si is not None:
        waits = list(si.on_wait)
        if len(waits) > MAX_WAITS:
            nc = self.nc
            overflow = waits[:-MAX_WAITS]
            for i in range(0, len(overflow), MAX_WAITS):
                chunk = overflow[i : i + MAX_WAITS]
                nop = mybir.InstNoOp(
                    name=nc.get_next_instruction_name(), engine=inst.engine
                )
                nop.sync_info = bass_rust.SyncInfo(on_wait=chunk, on_update=[])
                _orig_add_instruction(self, nop)
            si.on_wait = waits[-MAX_WAITS:]
    _orig_add_instruction(self, inst)


def _patched_drain_and_barrier(self, tick_clock, wait_clock):
    from concourse.tile import ScopedClock

    nc = self.nc
    # Pre-create nops to carry wait overflow (created before the drain so
    # they precede it in program order on the SP engine).
    nops = [nc.sync.nop(nofuse=True) for _ in range(40)]
    drain_inst = nc.sync.drain()
    wait_clock.add_sem_waits(
        drain_inst.ins, ScopedClock({None: tick_clock.global_clock})
    )
    si = drain_inst.ins.sync_info
    waits = list(si.on_wait) if si is not None else []
    if len(waits) > MAX_WAITS:
        chunks = [waits[i : i + MAX_WAITS] for i in range(0, len(waits), MAX_WAITS)]
        assert len(chunks) <= len(nops) + 1, "too many drain waits to split"
        # earlier chunks on the nops, the last chunk stays on the drain
        for nop_inst, chunk in zip(nops, chunks[:-1]):
            nsi = nop_inst.ins.sync_info
            if nsi is None:
                nop_inst.ins.sync_info = bass_rust.SyncInfo(
                    on_wait=chunk, on_update=[]
                )
            else:
                nsi.on_wait = chunk
        si.on_wait = chunks[-1]

    nc.all_engine_barrier()
    assert self.sems is not None
    popped = nc._tile_sem_poison_stack.pop()
    assert popped is self._sem_poison
    nc.clear_and_free_semaphores(list(self.sems.allocated().values()))
    nc.all_engine_barrier()


def install():
    global _installed
    if not _installed:
        tile.TileContext._drain_and_barrier = _patched_drain_and_barrier
        tile.TileContext._add_instruction = _patched_add_instruction
        _installed = True


def split_multi_waits(nc, limit=MAX_WAITS, verbose=False):
    """Safety net: move excess sync-waits onto the nearest preceding
    same-engine instruction with spare wait capacity (waiting earlier is
    always safe in straight-line code)."""
    n_moved = 0
    for f in nc.m.functions:
        for bb in f.blocks:
            insts = bb.instructions
            for i, inst in enumerate(insts):
                si = inst.sync_info
                if si is None:
                    continue
                waits = list(si.on_wait)
                if len(waits) <= limit:
                    continue
                excess = waits[:-limit]
                si.on_wait = waits[-limit:]
                j = i - 1
                while excess and j >= 0:
                    prev = insts[j]
                    if prev.engine == inst.engine:
                        psi = prev.sync_info
                        pw = list(psi.on_wait) if psi is not None else []
                        room = limit - len(pw)
                        if room > 0:
                            take = excess[-room:]
                            excess = excess[:-room]
                            neww = take + pw
                            if psi is None:
                                prev.sync_info = bass_rust.SyncInfo(
                                    on_wait=neww, on_update=[]
                                )
                            else:
                                psi.on_wait = neww
                            n_moved += len(take)
                    j -= 1
                if excess:
                    raise RuntimeError(
                        f"split_multi_waits: no room before {inst.name} in "
                        f"{bb.name} for {len(excess)} waits"
                    )
    if verbose and n_moved:
        print(f"split_multi_waits: moved {n_moved} waits")
    return n_moved

install()
# ---- end inlined workaround ----

FP32 = mybir.dt.float32
BF16 = mybir.dt.bfloat16
AF = mybir.ActivationFunctionType
ALU = mybir.AluOpType
AX = mybir.AxisListType

B, DIM, H, W = 4, 64, 88, 88
L = H * W                  # 7744
D, N, R, K = 32, 64, 2, 4
EPS = 1e-5
INV_CNT = 1.0 / (2 * B * L)
LT = [(i * 512, min(512, L - i * 512)) for i in range((L + 511) // 512)]
CHUNKS = [(0, 2048), (2048, 2048), (4096, 2048), (6144, 1600)]
NT = 16
QL = L // 4                # quarter-L for scratch-limited composites

_cache = {}


def _subtiles(off, ln):
    out = []
    p = 0
    while p < ln:
        w = min(512, ln - p)
        out.append((off + p, p, w))
        p += w
    return out


def build():
    nc = bass.Bass()

    def inp(name, shape):
        return nc.declare_dram_parameter(name, list(shape), FP32, isOutput=False)

    xsrc_d = inp("xsrc", (64, L))
    ysrc_d = inp("ysrc", (64, L))
    toxwT_d = inp("toxwT", (64, 32))
    toxb_d = inp("toxb", (32, 1))
    toywT_d = inp("toywT", (64, 64))
    toyb_d = inp("toyb", (64, 1))
    gx_d = inp("gx", (64, 1))
    bx_d = inp("bxp", (64, 1))
    gy_d = inp("gy", (64, 1))
    by_d = inp("byp", (64, 1))
    mrgb_d = inp("mrgb", (64, 1))
    xyBT_d = inp("xyBT", (32, 2 * 64))
    xyCT_d = inp("xyCT", (32, 2 * 64))
    wdtT_d = inp("wdtT", (32, 2 * 32))
    dtb_d = inp("dtb", (32, 2))
    convdiag_d = inp("convdiag", (32, 2 * 128))
    convb_d = inp("convb", (32, 2))
    dsk_d = inp("dsk", (32, 2))
    wfT_d = inp("wfT", (64, 64))
    projb_d = inp("projb", (64, 1))
    w1T_d = inp("w1T", (64, 4))
    w2T_d = inp("w2T", (4, 64))
    spw_d = inp("spw", (14, 7))
    n3g_d = inp("n3g", (128, 1))
    n3b_d = inp("n3b", (128, 1))
    cbrT_d = inp("cbrT", (128, 9 * 128))
    cbrg_d = inp("cbrg", (128, 1))
    cbrb_d = inp("cbrb", (128, 1))
    outwT_d = inp("outwT", (128, 128))
    outb_d = inp("outb", (128, 1))
    mlow_d = inp("mlow", (64, 1))
    acol_d = inp("acol", (128, 16))
    selB_d = inp("selB", (64, 16 * 128))
    sel32_d = inp("sel32", (128, 32))

    out_d = nc.declare_dram_parameter("out", [128, L], FP32, isOutput=True)

    ar1_in = nc.dram_tensor("ar1_in", [64, 4], FP32)
    ar1_out = nc.dram_tensor("ar1_out", [64, 4], FP32, addr_space="Shared")
    ar2_in = nc.dram_tensor("ar2_in", [64, 4], FP32)
    ar2_out = nc.dram_tensor("ar2_out", [64, 4], FP32, addr_space="Shared")
    ar3_in = nc.dram_tensor("ar3_in", [128, 2], FP32)
    ar3_out = nc.dram_tensor("ar3_out", [128, 2], FP32, addr_space="Shared")
    sp_bounce = nc.dram_tensor("sp_bounce", [1, L], FP32)
    ag_in = nc.dram_tensor("ag_in", [64, L], FP32)
    ag_out = nc.dram_tensor("ag_out", [128, L], FP32)
    GROUPS_ALL = [[0, 1, 2, 3, 4, 5, 6, 7]]
    GROUPS_PAIR = [[0, 4], [1, 5], [2, 6], [3, 7]]

    with tile.TileContext(nc) as tc, \
         tc.tile_pool(name="cols", bufs=1) as cp:

        carries = cp.tile([128, 16], FP32, tag="carries")

        acol = cp.tile([128, 16], FP32, tag="acol")
        nc.sync.dma_start(acol[:], acol_d[:])
        selBf = cp.tile([64, 16 * 128], FP32, tag="selBf")
        nc.sync.dma_start(selBf[:], selB_d[:])
        selK64 = cp.tile([64, 16 * 128], BF16, tag="selK64")
        nc.vector.tensor_copy(selK64[:], selBf[:])
        selK64hi = cp.tile([128, 16 * 128], BF16, tag="selK64hi")
        nc.vector.tensor_copy(selK64hi[64:128, :], selBf[:])
        sel32f = cp.tile([128, 32], FP32, tag="sel32f")
        nc.sync.dma_start(sel32f[:], sel32_d[:])
        sel32 = cp.tile([128, 32], BF16, tag="sel32")
        nc.vector.tensor_copy(sel32[:], sel32f[:])
        epscol = cp.tile([128, 1], FP32, tag="epscol")
        nc.gpsimd.memset(epscol[:], EPS)

        def silu_into(out_ap, in_ap, s1, s2):
            # silu(x) = x * exp(x - ln(1 + exp(x))); fp32 scratches s1, s2
            nc.scalar.activation(s1, in_ap, AF.Exp)
            nc.vector.tensor_scalar_add(s1, s1, 1.0)
            nc.scalar.activation(s1, s1, AF.Ln)
            nc.vector.tensor_sub(s2, in_ap, s1)
            nc.scalar.activation(s2, s2, AF.Exp)
            nc.vector.tensor_mul(out_ap, s2, in_ap)

        def sigmoid_into(out_ap, in_ap, s1):
            # in_ap must differ from out_ap
            nc.scalar.activation(s1, in_ap, AF.Exp)
            nc.vector.tensor_scalar_add(s1, s1, 1.0)
            nc.scalar.activation(s1, s1, AF.Ln)
            nc.vector.tensor_sub(out_ap, in_ap, s1)
            nc.scalar.activation(out_ap, out_ap, AF.Exp)

        def rsqrt_into(out_ap, in_ap, g_ap):
            p = in_ap.partition_size()
            nc.scalar.activation(out_ap, in_ap, AF.Ln, bias=epscol[:p, :])
            nc.scalar.activation(out_ap, out_ap, AF.Exp, scale=-0.5)
            nc.vector.tensor_mul(out_ap, out_ap, g_ap)

        # P_zy rows: 0:64 zyf (z 0:32, yf 32:64) -> vconv@0:32 -> ysF@0:32,
        # ysR@32:64; 64:96 sz; 96:128 x_proj.  Lives P1..P6.
        pmid_cm = tc.tile_pool(name="pmid", bufs=1)
        pmid = pmid_cm.__enter__()
        P_zy = pmid.tile([128, L], FP32, tag="P_zy")
        P_xc = pmid.tile([64, L], BF16, tag="P_xc")

        # scan-phase big operands, freed after P5
        with tc.tile_pool(name="psc", bufs=1) as psc:
            P_dt = psc.tile([64, L], FP32, tag="P_dt")
            P_BCF = psc.tile([128, L], BF16, tag="P_BCF")
            P_BCR = psc.tile([128, L], BF16, tag="P_BCR")

            # ================= P1+P2 =================
            with tc.tile_pool(name="pA", bufs=1) as pA, \
                 tc.tile_pool(name="pAs", bufs=2, space="PSUM") as psA:
                xs = pA.tile([64, L], FP32, tag="xs")
                ysr = pA.tile([64, L], FP32, tag="ysr")
                nc.sync.dma_start(xs[:], xsrc_d[:])
                nc.sync.dma_start(ysr[:], ysrc_d[:])
                scr = P_zy[64:128, :]  # scratch before sz/x_proj written
                st = cp.tile([64, 4], FP32, tag="st")
                nc.scalar.activation(scr, xs[:], AF.Identity,
                                     accum_out=st[:, 0:1])
                nc.scalar.activation(scr, xs[:], AF.Square,
                                     accum_out=st[:, 1:2])
                nc.scalar.activation(scr, ysr[:], AF.Identity,
                                     accum_out=st[:, 2:3])
                nc.scalar.activation(scr, ysr[:], AF.Square,
                                     accum_out=st[:, 3:4])
                mrgb = cp.tile([64, 1], FP32, tag="mrgb")
                nc.sync.dma_start(mrgb[:], mrgb_d[:])
                mdep = cp.tile([64, 1], FP32, tag="mdep")
                nc.vector.tensor_scalar(mdep[:], mrgb[:], -1.0, 1.0,
                                        ALU.mult, ALU.add)
                ar1s = cp.tile([64, 4], FP32, tag="ar1s")
                tmpc = cp.tile([64, 4], FP32, tag="tmpc")
                for j in range(2):
                    nc.vector.tensor_mul(ar1s[:, j:j+1], st[:, j:j+1], mrgb[:])
                    nc.vector.tensor_mul(tmpc[:, 0:1], st[:, 2+j:3+j], mdep[:])
                    nc.vector.tensor_add(ar1s[:, j:j+1], ar1s[:, j:j+1],
                                         tmpc[:, 0:1])
                    nc.vector.tensor_mul(ar1s[:, 2+j:3+j], st[:, j:j+1],
                                         mdep[:])
                    nc.vector.tensor_mul(tmpc[:, 1:2], st[:, 2+j:3+j], mrgb[:])
                    nc.vector.tensor_add(ar1s[:, 2+j:3+j], ar1s[:, 2+j:3+j],
                                         tmpc[:, 1:2])
                nc.sync.dma_start(ar1_in[:], ar1s[:])
                nc.gpsimd.collective_compute(
                    "AllReduce", ALU.add, replica_groups=GROUPS_ALL,
                    ins=[ar1_in[:]], outs=[ar1_out[:]])
                sums = cp.tile([64, 4], FP32, tag="sums")
                nc.sync.dma_start(sums[:], ar1_out[:])
                mv = cp.tile([64, 4], FP32, tag="mv")
                for j, (cs, cq) in enumerate(((0, 1), (2, 3))):
                    nc.vector.tensor_scalar_mul(mv[:, 2*j:2*j+1],
                                                sums[:, cs:cs+1], INV_CNT)
                    nc.vector.tensor_scalar_mul(mv[:, 2*j+1:2*j+2],
                                                sums[:, cq:cq+1], INV_CNT)
                    nc.vector.tensor_mul(tmpc[:, 2:3], mv[:, 2*j:2*j+1],
                                         mv[:, 2*j:2*j+1])
                    nc.vector.tensor_sub(mv[:, 2*j+1:2*j+2],
                                         mv[:, 2*j+1:2*j+2], tmpc[:, 2:3])
                sel = cp.tile([64, 4], FP32, tag="selstats")
                for j in range(2):
                    nc.vector.tensor_mul(sel[:, j:j+1], mv[:, j:j+1], mrgb[:])
                    nc.vector.tensor_mul(tmpc[:, 0:1], mv[:, 2+j:3+j], mdep[:])
                    nc.vector.tensor_add(sel[:, j:j+1], sel[:, j:j+1],
                                         tmpc[:, 0:1])
                    nc.vector.tensor_mul(sel[:, 2+j:3+j], mv[:, j:j+1],
                                         mdep[:])
                    nc.vector.tensor_mul(tmpc[:, 1:2], mv[:, 2+j:3+j],
                                         mrgb[:])
                    nc.vector.tensor_add(sel[:, 2+j:3+j], sel[:, 2+j:3+j],
                                         tmpc[:, 1:2])
                gx = cp.tile([64, 1], FP32, tag="gx")
                nc.sync.dma_start(gx[:], gx_d[:])
                bxp = cp.tile([64, 1], FP32, tag="bxp")
                nc.sync.dma_start(bxp[:], bx_d[:])
                gy = cp.tile([64, 1], FP32, tag="gy")
                nc.sync.dma_start(gy[:], gy_d[:])
                byp = cp.tile([64, 1], FP32, tag="byp")
                nc.sync.dma_start(byp[:], by_d[:])
                sxc = cp.tile([64, 1], FP32, tag="sxc")
                rsqrt_into(sxc[:], sel[:, 1:2], gx[:])
                bxc = cp.tile([64, 1], FP32, tag="bxc")
                nc.vector.tensor_mul(bxc[:], sel[:, 0:1], sxc[:])
                nc.vector.tensor_sub(bxc[:], bxp[:], bxc[:])
                syc = cp.tile([64, 1], FP32, tag="syc")
                rsqrt_into(syc[:], sel[:, 3:4], gy[:])
                byc = cp.tile([64, 1], FP32, tag="byc")
                nc.vector.tensor_mul(byc[:], sel[:, 2:3], syc[:])
                nc.vector.tensor_sub(byc[:], byp[:], byc[:])

                toxwT = cp.tile([64, 32], FP32, tag="toxwT")
                nc.sync.dma_start(toxwT[:], toxwT_d[:])
                toywT = cp.tile([64, 64], FP32, tag="toywT")
                nc.sync.dma_start(toywT[:], toywT_d[:])
                lhsx = cp.tile([64, 32], FP32, tag="lhsx")
                nc.vector.tensor_scalar_mul(lhsx[:], toxwT[:], sxc[:])
                lhsy = cp.tile([64, 64], FP32, tag="lhsy")
                nc.vector.tensor_scalar_mul(lhsy[:], toywT[:], syc[:])
                toxb = cp.tile([32, 1], FP32, tag="toxb")
                nc.sync.dma_start(toxb[:], toxb_d[:])
                toyb = cp.tile([64, 1], FP32, tag="toyb")
                nc.sync.dma_start(toyb[:], toyb_d[:])
                pbias = psA.tile([64, 1], FP32, tag="pbias")
                bxf = cp.tile([32, 1], FP32, tag="bxf")
                nc.tensor.matmul(pbias[:32, :], toxwT[:], bxc[:], start=True,
                                 stop=True)
                nc.scalar.activation(bxf[:], pbias[:32, :], AF.Identity,
                                     bias=toxb[:])
                byf = cp.tile([64, 1], FP32, tag="byf")
                nc.tensor.matmul(pbias[:], toywT[:], byc[:], start=True,
                                 stop=True)
                nc.scalar.activation(byf[:], pbias[:], AF.Identity,
                                     bias=toyb[:])

                # x_proj -> P_zy[96:128]; zyf -> P_zy[32:96]
                pmm = psA.tile([64, 512], FP32, tag="pmm")
                for (l0, w) in LT:
                    nc.tensor.matmul(pmm[:32, :w], lhsx[:], xs[:, l0:l0+w],
                                     start=True, stop=True)
                    nc.scalar.activation(P_zy[96:128, l0:l0+w], pmm[:32, :w],
                                         AF.Identity, bias=bxf[:])
                for (l0, w) in LT:
                    nc.tensor.matmul(pmm[:, :w], lhsy[:], ysr[:, l0:l0+w],
                                     start=True, stop=True)
                    nc.scalar.activation(P_zy[0:64, l0:l0+w], pmm[:, :w],
                                         AF.Identity, bias=byf[:])

                xyBT = cp.tile([32, 2 * 64], FP32, tag="xyBT")
                nc.sync.dma_start(xyBT[:], xyBT_d[:])
                xyCT = cp.tile([32, 2 * 64], FP32, tag="xyCT")
                nc.sync.dma_start(xyCT[:], xyCT_d[:])
                wdtT = cp.tile([32, 2 * 32], FP32, tag="wdtT")
                nc.sync.dma_start(wdtT[:], wdtT_d[:])
                # base-64 copies (matmul needs lhsT/rhs at same base partition)
                xyBTh = cp.tile([64, 2 * 64], FP32, tag="xyBTh")
                nc.sync.dma_start(xyBTh[32:64, :], xyBT_d[:])
                xyCTh = cp.tile([64, 2 * 64], FP32, tag="xyCTh")
                nc.sync.dma_start(xyCTh[32:64, :], xyCT_d[:])
                wdtTh = cp.tile([64, 2 * 32], FP32, tag="wdtTh")
                nc.sync.dma_start(wdtTh[32:64, :], wdtT_d[:])
                byf32 = cp.tile([32, 1], FP32, tag="byf32")
                nc.vector.tensor_copy(byf32[:], byf[32:64, :])
                dtbc = cp.tile([32, 2], FP32, tag="dtbc")
                nc.sync.dma_start(dtbc[:], dtb_d[:])
                bB = cp.tile([64, 2], FP32, tag="bB")
                bC = cp.tile([64, 2], FP32, tag="bC")
                bdt = cp.tile([32, 2], FP32, tag="bdt")
                yf = P_zy[32:64, :]
                for di in range(2):
                    nc.tensor.matmul(pbias[:], xyBT[:, di*64:(di+1)*64],
                                     byf32[:], start=True, stop=True)
                    nc.scalar.copy(bB[:, di:di+1], pbias[:])
                    nc.tensor.matmul(pbias[:], xyCT[:, di*64:(di+1)*64],
                                     byf32[:], start=True, stop=True)
                    nc.scalar.copy(bC[:, di:di+1], pbias[:])
                    nc.tensor.matmul(pbias[:32, :], wdtT[:, di*32:(di+1)*32],
                                     byf32[:], start=True, stop=True)
                    nc.scalar.activation(bdt[:, di:di+1], pbias[:32, :],
                                         AF.Identity, bias=dtbc[:, di:di+1])
                    dstBC = P_BCF if di == 0 else P_BCR
                    dstB = dstBC[0:64, :]
                    dstC = dstBC[64:128, :]
                    # Bm / Cm: write unreversed; R reversed later
                    for (l0, w) in LT:
                        nc.tensor.matmul(pmm[:, :w],
                                         xyBTh[32:64, di*64:(di+1)*64],
                                         yf[:, l0:l0+w], start=True, stop=True)
                        nc.scalar.activation(dstB[:, l0:l0+w], pmm[:, :w],
                                             AF.Identity, bias=bB[:, di:di+1])
                    for (l0, w) in LT:
                        nc.tensor.matmul(pmm[:, :w],
                                         xyCTh[32:64, di*64:(di+1)*64],
                                         yf[:, l0:l0+w], start=True, stop=True)
                        nc.scalar.activation(dstC[:, l0:l0+w], pmm[:, :w],
                                             AF.Identity, bias=bC[:, di:di+1])
                    # dt: softplus = ln(1+exp(lin+b)) per half to bound scratch
                    drow = P_dt[di*32:(di+1)*32, :]
                    for (l0, w) in LT:
                        nc.tensor.matmul(pmm[:32, :w],
                                         wdtTh[32:64, di*32:(di+1)*32],
                                         yf[:, l0:l0+w], start=True, stop=True)
                        nc.scalar.activation(drow[:, l0:l0+w], pmm[:32, :w],
                                             AF.Exp, bias=bdt[:, di:di+1])
                    nc.vector.tensor_scalar_add(drow, drow, 1.0)
                    nc.scalar.activation(drow, drow, AF.Ln)

            # reverse R halves in place via scratch quarters
            with tc.tile_pool(name="prev", bufs=1) as pv:
                rq = pv.tile([64, QL], FP32, tag="rq")
                for q in range(2):
                    a0, b0 = q * QL, L - (q + 1) * QL
                    nc.vector.tensor_copy(rq[0:32, :], P_dt[32:64, a0:a0+QL])
                    nc.vector.tensor_copy(rq[32:64, :], P_dt[32:64, b0:b0+QL])
                    nc.vector.tensor_copy(P_dt[32:64, a0:a0+QL],
                                          rq[32:64, ::-1])
                    nc.vector.tensor_copy(P_dt[32:64, b0:b0+QL],
                                          rq[0:32, ::-1])
                rq16 = pv.tile([128, QL], BF16, tag="rq16")
                for q in range(2):
                    a0, b0 = q * QL, L - (q + 1) * QL
                    nc.vector.tensor_copy(rq16[:, :], P_BCR[:, a0:a0+QL])
                    nc.vector.tensor_copy(P_BCR[:, a0:a0+QL],
                                          P_BCR[:, b0:b0+QL][:, ::-1])
                    nc.vector.tensor_copy(P_BCR[:, b0:b0+QL], rq16[:, ::-1])

            # ============ P3: sz, conv1d+silu ============
            with tc.tile_pool(name="pB", bufs=1) as pB, \
                 tc.tile_pool(name="pBs", bufs=2, space="PSUM") as psB:
                s1 = pB.tile([32, QL], FP32, tag="s1")
                s2 = pB.tile([32, QL], FP32, tag="s2")
                # sz = silu(z): z in P_zy[0:32]; out P_zy[64:96]
                for q in range(4):
                    a0 = q * QL
                    silu_into(P_zy[64:96, a0:a0+QL], P_zy[0:32, a0:a0+QL],
                              s1[:], s2[:])
                convdiag = cp.tile([32, 2 * 128], FP32, tag="convdiag")
                nc.sync.dma_start(convdiag[:], convdiag_d[:])
                convdiag16 = cp.tile([32, 2 * 128], BF16, tag="convdiag16")
                nc.vector.tensor_copy(convdiag16[:], convdiag[:])
                convb = cp.tile([32, 2], FP32, tag="convb")
                nc.sync.dma_start(convb[:], convb_d[:])
                xpad = pB.tile([32, L + 3], BF16, tag="xpad")
                pc = psB.tile([32, 512], FP32, tag="pc")
                for di in range(2):
                    nc.gpsimd.memset(xpad[:, 0:3], 0.0)
                    if di == 0:
                        nc.vector.tensor_copy(xpad[:, 3:3+L], P_zy[96:128, :])
                    else:
                        nc.vector.tensor_copy(xpad[:, 3:3+L],
                                              P_zy[96:128, ::-1])
                    # conv into vconv = P_zy[0:32] (z dead after sz)
                    vconv = P_zy[0:32, :]
                    for (l0, w) in LT:
                        for k in range(4):
                            nc.tensor.matmul(
                                pc[:, :w],
                                convdiag16[:, di*128+k*32:di*128+(k+1)*32],
                                xpad[:, l0+k:l0+k+w],
                                start=(k == 0), stop=(k == 3))
                        nc.scalar.activation(vconv[:, l0:l0+w], pc[:, :w],
                                             AF.Identity,
                                             bias=convb[:, di:di+1])
                    for q in range(4):
                        a0 = q * QL
                        silu_into(P_xc[di*32:(di+1)*32, a0:a0+QL],
                                  vconv[:, a0:a0+QL], s1[:], s2[:])

            # ================= P5: scans =================
            with tc.tile_pool(name="scp", bufs=2) as sp_, \
                 tc.tile_pool(name="repp", bufs=1) as rp_, \
                 tc.tile_pool(name="scps", bufs=1, space="PSUM") as reps, \
                 tc.tile_pool(name="ysps", bufs=4, space="PSUM") as ysps:
                for di in range(2):
                    BC = P_BCF if di == 0 else P_BCR
                    for (c0, cw) in CHUNKS:
                        dt_rep = rp_.tile([128, 2048], FP32, tag="dt_rep")
                        dtxc_rep = rp_.tile([128, 2048], BF16, tag="dtxc_rep")
                        nc.vector.tensor_mul(
                            dtxc_rep[0:32, :cw], P_dt[di*32:(di+1)*32, c0:c0+cw],
                            P_xc[di*32:(di+1)*32, c0:c0+cw])
                        for q in range(1, 4):
                            nc.gpsimd.tensor_copy(dtxc_rep[32*q:32*(q+1), :cw],
                                                  dtxc_rep[0:32, :cw])
                            nc.gpsimd.tensor_copy(
                                dt_rep[32*q:32*(q+1), :cw],
                                P_dt[di*32:(di+1)*32, c0:c0+cw])
                        nc.gpsimd.tensor_copy(dt_rep[0:32, :cw],
                                              P_dt[di*32:(di+1)*32, c0:c0+cw])
                        ys_subs = []
                        for (a0, p0, w) in _subtiles(c0, cw):
                            yst = ysps.tile([32, 512], FP32, tag="ys")
                            ys_subs.append((yst, a0, p0, w))
                        for t in range(NT):
                            dA = sp_.tile([128, 2048], FP32, tag="dA")
                            nc.scalar.activation(dA[:, :cw], dt_rep[:, :cw],
                                                 AF.Exp, scale=acol[:, t:t+1])
                            rep = reps.tile([128, 2048], FP32, tag="rep")
                            lsl = selK64[:, t * 128:(t + 1) * 128]
                            lslC = selK64hi[64:128, t * 128:(t + 1) * 128]
                            for (a0, p0, w) in _subtiles(c0, cw):
                                nc.tensor.matmul(rep[:, p0:p0+w], lsl,
                                                 BC[0:64, a0:a0+w],
                                                 start=True, stop=True)
                            dBu = sp_.tile([128, 2048], BF16, tag="dBu")
                            nc.vector.tensor_mul(dBu[:, :cw], rep[:, :cw],
                                                 dtxc_rep[:, :cw])
                            hs = sp_.tile([128, 2048], FP32, tag="hs")
                            init = 0.0 if c0 == 0 else carries[:, t:t+1]
                            nc.vector.tensor_tensor_scan(
                                hs[:, :cw], dA[:, :cw], dBu[:, :cw], init,
                                ALU.mult, ALU.add)
                            nc.gpsimd.tensor_copy(carries[:, t:t+1],
                                                   hs[:, cw-1:cw])
                            rep2 = reps.tile([128, 2048], FP32, tag="rep")
                            for (a0, p0, w) in _subtiles(c0, cw):
                                nc.tensor.matmul(rep2[:, p0:p0+w], lslC,
                                                 BC[64:128, a0:a0+w],
                                                 start=True, stop=True)
                            hc = sp_.tile([128, 2048], BF16, tag="hc")
                            nc.vector.tensor_mul(hc[:, :cw], hs[:, :cw],
                                                 rep2[:, :cw])
                            for (yst, a0, p0, w) in ys_subs:
                                nc.tensor.matmul(yst[:, :w], sel32,
                                                 hc[:, p0:p0+w],
                                                 start=(t == 0),
                                                 stop=(t == NT - 1))
                        for (yst, a0, p0, w) in ys_subs:
                            nc.scalar.copy(
                                P_zy[di*32:32 + di*32, a0:a0+w], yst[:, :w])

        # ============ P6: combine + proj + resid + CBAM ============
        with tc.tile_pool(name="p6", bufs=1) as p6, \
             tc.tile_pool(name="p6small", bufs=2) as p6s, \
             tc.tile_pool(name="p6ps", bufs=2, space="PSUM") as ps6:
            dsk = cp.tile([32, 2], FP32, tag="dsk")
            nc.sync.dma_start(dsk[:], dsk_d[:])
            dsk32b = cp.tile([64, 2], FP32, tag="dsk32b")
            nc.sync.dma_start(dsk32b[32:64, :], dsk_d[:])
            outFR = p6.tile([64, L], FP32, tag="outFR")
            # ys += Dsk*xc; out = ys*silu(z)  (R still in reversed time)
            # (2-input DVE ops need equal input base partitions -> use a
            #  (64,512) staging tile and operate at matching row offsets)
            for di in range(2):
                r0 = di * 32
                yrow = P_zy[r0:r0+32, :]
                for (l0, w) in LT:
                    tmp = p6s.tile([64, 512], FP32, tag="tmp64")
                    nc.vector.tensor_scalar_mul(
                        tmp[r0:r0+32, :w], P_xc[r0:r0+32, l0:l0+w],
                        dsk[:32, di:di+1] if di == 0 else dsk32b[32:64, di:di+1])
                    nc.vector.tensor_add(yrow[:, l0:l0+w], yrow[:, l0:l0+w],
                                         tmp[r0:r0+32, :w])
            for (l0, w) in LT:
                tmp = p6s.tile([64, 512], FP32, tag="tmp64")
                nc.vector.tensor_copy(tmp[0:32, :w], P_zy[64:96, l0:l0+w])
                nc.vector.tensor_mul(outFR[0:32, l0:l0+w],
                                     P_zy[0:32, l0:l0+w], tmp[0:32, :w])
            # outR: multiply reversed-time ysR by reversed sz, then unreverse
            for (l0, w) in LT:
                tmp = p6s.tile([64, 512], FP32, tag="tmp64")
                nc.vector.tensor_copy(tmp[32:64, :w],
                                      P_zy[64:96, L-l0-w:L-l0][:, ::-1])
                nc.vector.tensor_mul(tmp[32:64, :w], P_zy[32:64, l0:l0+w],
                                     tmp[32:64, :w])
                nc.vector.tensor_copy(outFR[32:64, L-l0-w:L-l0],
                                      tmp[32:64, :w][:, ::-1])
            wfT = cp.tile([64, 64], FP32, tag="wfT")
            nc.sync.dma_start(wfT[:], wfT_d[:])
            projb = cp.tile([64, 1], FP32, tag="projb")
            nc.sync.dma_start(projb[:], projb_d[:])
            x2 = p6.tile([64, L], FP32, tag="x2")
            pm6 = ps6.tile([64, 512], FP32, tag="pm6")
            for (l0, w) in LT:
                nc.tensor.matmul(pm6[:, :w], wfT[:], outFR[:, l0:l0+w],
                                 start=True, stop=True)
                nc.scalar.activation(x2[:, l0:l0+w], pm6[:, :w], AF.Identity,
                                     bias=projb[:])
            for (l0, w) in LT:
                rt = p6s.tile([64, 512], FP32, tag="rt")
                nc.sync.dma_start(rt[:, :w], xsrc_d[:, l0:l0+w])
                nc.vector.tensor_add(x2[:, l0:l0+w], x2[:, l0:l0+w],
                                     rt[:, :w])

            # channel attention
            colA = cp.tile([64, 1], FP32, tag="colA")
            colB = cp.tile([64, 1], FP32, tag="colB")
            nc.vector.tensor_reduce(colA[:], x2[:], AX.X, ALU.add)
            nc.vector.tensor_scalar_mul(colA[:], colA[:], 1.0 / L)
            nc.vector.tensor_reduce(colB[:], x2[:], AX.X, ALU.max)
            w1T = cp.tile([64, 4], FP32, tag="w1T")
            nc.sync.dma_start(w1T[:], w1T_d[:])
            w2T = cp.tile([4, 64], FP32, tag="w2T")
            nc.sync.dma_start(w2T[:], w2T_d[:])
            pml = ps6.tile([4, 1], FP32, tag="small")
            rl = cp.tile([4, 2], FP32, tag="rl")
            nc.tensor.matmul(pml[:], w1T[:], colA[:], start=True, stop=True)
            nc.scalar.activation(rl[:, 0:1], pml[:], AF.Relu)
            nc.tensor.matmul(pml[:], w1T[:], colB[:], start=True, stop=True)
            nc.scalar.activation(rl[:, 1:2], pml[:], AF.Relu)
            pca = ps6.tile([64, 1], FP32, tag="small")
            nc.tensor.matmul(pca[:], w2T[:], rl[:, 0:1], start=True,
                             stop=False)
            nc.tensor.matmul(pca[:], w2T[:], rl[:, 1:2], start=False,
                             stop=True)
            cac = cp.tile([64, 1], FP32, tag="cac")
            ct0 = cp.tile([64, 1], FP32, tag="ct0")
            ct1 = cp.tile([64, 1], FP32, tag="ct1")
            nc.scalar.copy(ct0[:], pca[:])
            sigmoid_into(cac[:], ct0[:], ct1[:])
            nc.vector.tensor_scalar_mul(x2[:], x2[:], cac[:])

            # spatial attention: stack rows (c,i), layout (88, 94) per row
            SW = 94
            stack = p6.tile([14, H * SW], BF16, tag="stack")
            nc.gpsimd.memset(stack[:], 0.0)
            ones64 = cp.tile([64, 1], FP32, tag="ones64")
            nc.gpsimd.memset(ones64[:], 1.0)
            HG = [(h0, min(5, H - h0)) for h0 in range(0, H, 5)]
            st3d = stack[:, :].rearrange("p (h w) -> p h w", w=SW)
            pg = ps6.tile([1, 512], FP32, tag="small")
            # mean row -> stack row 3 ; max row -> stack row 10
            # (engine outputs must sit at base partition 0; DMA into rows)
            for (h0, hc_) in HG:
                nc.tensor.matmul(pg[:, :hc_*W], ones64[:],
                                 x2[:, h0*W:(h0+hc_)*W], start=True, stop=True)
                srm = p6s.tile([1, 512], FP32, tag="srm")
                nc.scalar.activation(srm[:, :hc_*W], pg[:, :hc_*W], AF.Copy,
                                     scale=1.0 / 64)
                nc.gpsimd.dma_start(
                    st3d[3:4, h0:h0+hc_, 3:3+W],
                    srm[:, :hc_*W].rearrange("p (h w) -> p h w", w=W))
                srx = p6s.tile([1, 512], FP32, tag="srx")
                nc.gpsimd.tensor_reduce(
                    srx[:, :hc_*W],
                    x2[:, h0*W:(h0+hc_)*W], AX.C, ALU.max)
                nc.gpsimd.dma_start(
                    st3d[10:11, h0:h0+hc_, 3:3+W],
                    srx[:, :hc_*W].rearrange("p (h w) -> p h w", w=W))
            # shifted copies for i != 3
            for c_ in range(2):
                src_r = 3 if c_ == 0 else 10
                for i_ in range(7):
                    r = c_ * 7 + i_
                    if i_ == 3:
                        continue
                    sh = i_ - 3
                    h_lo = max(0, -sh)
                    h_hi = min(H, H - sh)
                    nc.sync.dma_start(
                        st3d[r:r+1, h_lo:h_hi, 3:3+W],
                        st3d[src_r:src_r+1, h_lo+sh:h_hi+sh, 3:3+W])
            spw = cp.tile([14, 7], FP32, tag="spw")
            nc.sync.dma_start(spw[:], spw_d[:])
            spw16 = cp.tile([14, 7], BF16, tag="spw16")
            nc.vector.tensor_copy(spw16[:], spw[:])
            psa = ps6.tile([1, 512], FP32, tag="small")
            srow = p6s.tile([1, 512], FP32, tag="srow")
            for (h0, hc_) in HG:
                for jj in range(7):
                    rhs = st3d[:, h0:h0+hc_, jj:jj+W]
                    nc.tensor.matmul(psa[:, :hc_*W], spw16[:, jj:jj+1], rhs,
                                     start=(jj == 0), stop=(jj == 6))
                srow = p6s.tile([1, 512], FP32, tag="srow")
                nc.scalar.copy(srow[:, :hc_*W], psa[:, :hc_*W])
                nc.sync.dma_start(sp_bounce[0:1, h0*W:(h0+hc_)*W],
                                  srow[:, :hc_*W])
            sa2d = p6s.tile([88, 88], FP32, tag="sa2d")
            nc.sync.dma_start(sa2d[:],
                              sp_bounce[0, :].rearrange("(h w) -> h w", w=W))
            s2a = p6s.tile([88, 88], FP32, tag="s2a")
            s2c = p6s.tile([88, 88], FP32, tag="s2c")
            sigmoid_into(s2c[:], sa2d[:], s2a[:])
            nc.sync.dma_start(sp_bounce[0, :].rearrange("(h w) -> h w", w=W),
                              s2c[:])
            ones1 = cp.tile([1, 64], FP32, tag="ones1")
            nc.gpsimd.memset(ones1[:], 1.0)
            for (l0, w) in LT:
                sarow = p6s.tile([1, 512], FP32, tag="sarow")
                nc.sync.dma_start(sarow[:, :w], sp_bounce[:, l0:l0+w])
                pbc = ps6.tile([64, 512], FP32, tag="pm6")
                nc.tensor.matmul(pbc[:, :w], ones1[:], sarow[:, :w],
                                 start=True, stop=True)
                nc.vector.tensor_mul(outFR[:, l0:l0+w], x2[:, l0:l0+w],
                                     pbc[:, :w])
            x3 = outFR  # reuse

            nc.sync.dma_start(ag_in[:], x3[:])
            st2 = cp.tile([64, 2], FP32, tag="st2")
            nc.scalar.activation(x2[:], x3[:], AF.Identity,
                                 accum_out=st2[:, 0:1])
            nc.scalar.activation(x2[:], x3[:], AF.Square,
                                 accum_out=st2[:, 1:2])
            mlow = cp.tile([64, 1], FP32, tag="mlow")
            nc.sync.dma_start(mlow[:], mlow_d[:])
            mhigh = cp.tile([64, 1], FP32, tag="mhigh")
            nc.vector.tensor_scalar(mhigh[:], mlow[:], -1.0, 1.0, ALU.mult,
                                    ALU.add)
            ar2s = cp.tile([64, 4], FP32, tag="ar2s")
            for j in range(2):
                nc.vector.tensor_mul(ar2s[:, j:j+1], st2[:, j:j+1], mlow[:])
                nc.vector.tensor_mul(ar2s[:, 2+j:3+j], st2[:, j:j+1],
                                     mhigh[:])
            nc.sync.dma_start(ar2_in[:], ar2s[:])
            nc.gpsimd.collective_compute(
                "AllReduce", ALU.add, replica_groups=GROUPS_ALL,
                ins=[ar2_in[:]], outs=[ar2_out[:]])
            nc.gpsimd.collective_compute(
                "AllGather", ALU.bypass, replica_groups=GROUPS_PAIR,
                ins=[ag_in[:]], outs=[ag_out[:]])

        pmid_cm.__exit__(None, None, None)

        # ============ P9: FFN ============
        with tc.tile_pool(name="p9", bufs=1) as p9, \
             tc.tile_pool(name="p9s", bufs=2, space="PSUM") as ps9:
            rgbd = p9.tile([128, L], FP32, tag="rgbd")
            nc.sync.dma_start(rgbd[:], ag_out[:])
            s2st = cp.tile([64, 4], FP32, tag="s2st")
            nc.sync.dma_start(s2st[:], ar2_out[:])
            n3g = cp.tile([128, 1], FP32, tag="n3g")
            nc.sync.dma_start(n3g[:], n3g_d[:])
            n3b = cp.tile([128, 1], FP32, tag="n3b")
            nc.sync.dma_start(n3b[:], n3b_d[:])
            sc128 = cp.tile([128, 1], FP32, tag="sc128")
            sh128 = cp.tile([128, 1], FP32, tag="sh128")
            mvt = cp.tile([64, 2], FP32, tag="mvt")
            ctA = cp.tile([64, 1], FP32, tag="ctA")
            gh = cp.tile([64, 1], FP32, tag="gh")
            bh = cp.tile([64, 1], FP32, tag="bh")
            sch = cp.tile([64, 1], FP32, tag="sch")
            shh = cp.tile([64, 1], FP32, tag="shh")
            for half in range(2):
                r0 = half * 64
                nc.vector.tensor_copy(gh[:], n3g[r0:r0+64, :])
                nc.vector.tensor_copy(bh[:], n3b[r0:r0+64, :])
                nc.vector.tensor_scalar_mul(mvt[:, 0:1],
                                            s2st[:, 2*half:2*half+1], INV_CNT)
                nc.vector.tensor_scalar_mul(mvt[:, 1:2],
                                            s2st[:, 2*half+1:2*half+2],
                                            INV_CNT)
                nc.vector.tensor_mul(ctA[:], mvt[:, 0:1], mvt[:, 0:1])
                nc.vector.tensor_sub(mvt[:, 1:2], mvt[:, 1:2], ctA[:])
                rsqrt_into(sch[:], mvt[:, 1:2], gh[:])
                nc.vector.tensor_mul(ctA[:], mvt[:, 0:1], sch[:])
                nc.vector.tensor_sub(shh[:], bh[:], ctA[:])
                nc.vector.tensor_copy(sc128[r0:r0+64, :], sch[:])
                nc.vector.tensor_copy(sh128[r0:r0+64, :], shh[:])
            n316 = p9.tile([128, L], BF16, tag="n316")
            nc.vector.tensor_scalar(n316[:], rgbd[:], sc128[:], sh128[:],
                                    ALU.mult, ALU.add)
            PW = 90
            n3pad = p9.tile([128, 90 * PW], BF16, tag="n3pad")
            nc.gpsimd.memset(n3pad[:], 0.0)
            nc.sync.dma_start(
                n3pad[:, :].rearrange("p (h w) -> p h w", w=PW)[:, 1:89, 1:89],
                n316[:, :].rearrange("p (h w) -> p h w", w=W))
            cbrT16 = p9.tile([128, 9 * 128], BF16, tag="cbrT16")
            with tc.tile_pool(name="pcl", bufs=1) as pcl:
                cbrT = pcl.tile([128, 9 * 128], FP32, tag="cbrT")
                nc.sync.dma_start(cbrT[:], cbrT_d[:])
                nc.vector.tensor_copy(cbrT16[:], cbrT[:])
            h3 = p9.tile([128, L], FP32, tag="h3")
            pc9 = ps9.tile([128, 440], FP32, tag="pc9")
            HG = [(h0, min(5, H - h0)) for h0 in range(0, H, 5)]
            n3p3 = n3pad[:, :].rearrange("p (h w) -> p h w", w=PW)
            for (h0, hc_) in HG:
                for ij in range(9):
                    i_, j_ = ij // 3, ij % 3
                    rhs = n3p3[:, h0+i_:h0+i_+hc_, j_:j_+W]
                    nc.tensor.matmul(pc9[:, :hc_*W],
                                     cbrT16[:, ij*128:(ij+1)*128], rhs,
                                     start=(ij == 0), stop=(ij == 8))
                nc.scalar.copy(h3[:, h0*W:(h0+hc_)*W], pc9[:, :hc_*W])
            st3 = cp.tile([128, 2], FP32, tag="st3")
            hr = p9.tile([128, L], FP32, tag="hr")
            nc.scalar.activation(hr[:], h3[:], AF.Identity,
                                 accum_out=st3[:, 0:1])
            nc.scalar.activation(hr[:], h3[:], AF.Square,
                                 accum_out=st3[:, 1:2])
            nc.sync.dma_start(ar3_in[:], st3[:])
            nc.gpsimd.collective_compute(
                "AllReduce", ALU.add, replica_groups=GROUPS_ALL,
                ins=[ar3_in[:]], outs=[ar3_out[:]])
            st3o = cp.tile([128, 2], FP32, tag="st3o")
            nc.sync.dma_start(st3o[:], ar3_out[:])
            cbrg = cp.tile([128, 1], FP32, tag="cbrg")
            nc.sync.dma_start(cbrg[:], cbrg_d[:])
            cbrb = cp.tile([128, 1], FP32, tag="cbrb")
            nc.sync.dma_start(cbrb[:], cbrb_d[:])
            m3c = cp.tile([128, 1], FP32, tag="m3c")
            v3c = cp.tile([128, 1], FP32, tag="v3c")
            ct3 = cp.tile([128, 1], FP32, tag="ct3")
            nc.vector.tensor_scalar_mul(m3c[:], st3o[:, 0:1], INV_CNT)
            nc.vector.tensor_scalar_mul(v3c[:], st3o[:, 1:2], INV_CNT)
            nc.vector.tensor_mul(ct3[:], m3c[:], m3c[:])
            nc.vector.tensor_sub(v3c[:], v3c[:], ct3[:])
            sc3 = cp.tile([128, 1], FP32, tag="sc3")
            rsqrt_into(sc3[:], v3c[:], cbrg[:])
            sh3 = cp.tile([128, 1], FP32, tag="sh3")
            nc.vector.tensor_mul(sh3[:], m3c[:], sc3[:])
            nc.vector.tensor_sub(sh3[:], cbrb[:], sh3[:])
            nc.vector.tensor_scalar(hr[:], h3[:], sc3[:], sh3[:], ALU.mult,
                                    ALU.add)
            nc.scalar.activation(hr[:], hr[:], AF.Relu)
            outwT = cp.tile([128, 128], FP32, tag="outwT")
            nc.sync.dma_start(outwT[:], outwT_d[:])
            outb = cp.tile([128, 1], FP32, tag="outb")
            nc.sync.dma_start(outb[:], outb_d[:])
            fin = h3  # reuse
            pf = ps9.tile([128, 512], FP32, tag="pf")
            for (l0, w) in LT:
                nc.tensor.matmul(pf[:, :w], outwT[:], hr[:, l0:l0+w],
                                 start=True, stop=True)
                nc.scalar.activation(fin[:, l0:l0+w], pf[:, :w], AF.Identity,
                                     bias=outb[:])
            nc.vector.tensor_add(fin[:], fin[:], rgbd[:])
            nc.sync.dma_start(out_d[:], fin[:])

    return nc


def _host_prep(rgb, depth, params):
    rgbf = np.ascontiguousarray(np.asarray(rgb, np.float32).reshape(B, DIM, L))
    depf = np.ascontiguousarray(np.asarray(depth, np.float32).reshape(B, DIM, L))

    def n32(x):
        return np.ascontiguousarray(np.asarray(x, np.float32))

    acol = np.zeros((128, 16), np.float32)
    for t in range(16):
        for row in range(128):
            acol[row, t] = -(4 * t + row // 32 + 1)
    selB = np.zeros((64, 16 * 128), np.float32)
    for t in range(16):
        for m in range(128):
            selB[4 * t + m // 32, t * 128 + m] = 1.0
    sel32 = np.zeros((128, 32), np.float32)
    for row in range(128):
        sel32[row, row % 32] = 1.0

    in_maps = []
    for c in range(8):
        blk, b = c // 4, c % 4
        p = params['d2r'] if blk == 0 else params['r2d']
        pcb = params['cbam1'] if blk == 0 else params['cbam2']
        if blk == 0:
            xs, ys_ = rgbf[b], depf[b]
            gx, bx = n32(params['rgb_g']), n32(params['rgb_b'])
            gy, by = n32(params['dep_g']), n32(params['dep_b'])
            mrgb = 1.0
        else:
            xs, ys_ = depf[b], rgbf[b]
            gx, bx = n32(params['dep_g']), n32(params['dep_b'])
            gy, by = n32(params['rgb_g']), n32(params['rgb_b'])
            mrgb = 0.0
        mF, mR = p['mF'], p['mR']
        xyBT = np.concatenate(
            [n32(m['xy_w'])[R:R+N].T for m in (mF, mR)], 1)
        xyCT = np.concatenate(
            [n32(m['xy_w'])[R+N:].T for m in (mF, mR)], 1)
        wdtT = np.concatenate(
            [(n32(m['dt_w']) @ n32(m['xy_w'])[:R]).T for m in (mF, mR)], 1)
        dtb = np.stack([n32(m['dt_b']) for m in (mF, mR)], 1)
        convdiag = np.concatenate(
            [np.concatenate([np.diag(n32(m['conv_w'])[:, k]) for k in range(4)],
                            1) for m in (mF, mR)], 1)
        convb = np.stack([n32(m['conv_b']) for m in (mF, mR)], 1)
        dsk = np.stack([n32(m['Dsk']) for m in (mF, mR)], 1)
        bd = np.zeros((64, 64), np.float32)
        bd[:32, :32] = n32(mF['out_w'])
        bd[32:, 32:] = n32(mR['out_w'])
        wf = n32(p['proj_w']) @ bd
        ffn = params['ffn']
        cbrT = np.concatenate(
            [n32(ffn['cbr_w'])[:, :, ij // 3, ij % 3].T for ij in range(9)], 1)
        spw = n32(pcb['sp_w'])[0].reshape(14, 7)
        in_maps.append({
            "xsrc": xs, "ysrc": ys_,
            "toxwT": n32(p['to_x_w']).T.copy(), "toxb": n32(p['to_x_b'])[:, None],
            "toywT": n32(p['to_y_w']).T.copy(), "toyb": n32(p['to_y_b'])[:, None],
            "gx": gx[:, None], "bxp": bx[:, None],
            "gy": gy[:, None], "byp": by[:, None],
            "mrgb": np.full((64, 1), mrgb, np.float32),
            "xyBT": np.ascontiguousarray(xyBT),
            "xyCT": np.ascontiguousarray(xyCT),
            "wdtT": np.ascontiguousarray(wdtT),
            "dtb": np.ascontiguousarray(dtb),
            "convdiag": np.ascontiguousarray(convdiag),
            "convb": np.ascontiguousarray(convb),
            "dsk": np.ascontiguousarray(dsk),
            "wfT": np.ascontiguousarray(wf.T.copy()),
            "projb": n32(p['proj_b'])[:, None],
            "w1T": n32(pcb['w1']).T.copy(), "w2T": n32(pcb['w2']).T.copy(),
            "spw": np.ascontiguousarray(spw),
            "n3g": n32(params['n3_g'])[:, None],
            "n3b": n32(params['n3_b'])[:, None],
            "cbrT": np.ascontiguousarray(cbrT),
            "cbrg": n32(ffn['cbr_g'])[:, None],
            "cbrb": n32(ffn['cbr_b'])[:, None],
            "outwT": n32(ffn['out_w']).T.copy(),
            "outb": n32(ffn['out_b'])[:, None],
            "mlow": np.full((64, 1), 1.0 if blk == 0 else 0.0, np.float32),
            "acol": acol, "selB": selB, "sel32": sel32,
        })
    return in_maps


def kernel(rgb, depth, params):
    if "nc" not in _cache:
        _cache["nc"] = build()
    nc = _cache["nc"]
    in_maps = _host_prep(rgb, depth, params)
    res = bass_utils.run_bass_kernel_spmd(nc, in_maps, list(range(8)))
    out = np.stack([res.results[b]["out"].reshape(128, H, W)
                    for b in range(B)])
    return out.astype(np.float32)


if __name__ == "__main__":
    import jax
    jax.config.update('jax_platforms', 'cpu')
    import reference as ref
    inputs = ref.setup_inputs()
    expected = np.asarray(ref.reference(**inputs))
    got = kernel(np.asarray(inputs['rgb']), np.asarray(inputs['depth']),
                 inputs['params'])
    err = np.abs(got - expected)
    den = np.abs(expected).mean()
    print("max abs err:", err.max(), "mean rel:", err.mean() / den)


# revision 17
# speedup vs baseline: 1.0296x; 1.0296x over previous
"""TRN2 Bass kernel for nn_CMDB: cross-modal Mamba(S6) fusion block.

Sharding: 8 cores; core c handles blk = c//4 (0: d2r, 1: r2d), sample
b = c%4. Each core runs the full cmfb+CBAM for its (blk, b) half, then the
pair exchanges halves (AllGather) and both redundantly compute the final
FFN for sample b; host takes cores 0-3 outputs.

Self-contained: hardcodes all shapes; host does only slicing/layout prep of
weights (data-independent) and shard assembly.
"""
import sys
import numpy as np

sys.path.insert(0, "/root/problem")
sys.path.insert(0, "/opt/trn_rl_repo")

import concourse.bass as bass
import concourse.mybir as mybir
import concourse.tile as tile
from concourse import bass_utils

# ---- inlined walrus single-wait workaround (see tile_fixups.py) ----


import bass_rust

MAX_WAITS = 1

_installed = False
_orig_add_instruction = tile.TileContext._add_instruction


def _patched_add_instruction(self, inst):
    si = inst.sync_info
    if si is not None:
        waits = list(si.on_wait)
        if len(waits) > MAX_WAITS:
            nc = self.nc
            overflow = waits[:-MAX_WAITS]
            for i in range(0, len(overflow), MAX_WAITS):
                chunk = overflow[i : i + MAX_WAITS]
                nop = mybir.InstNoOp(
                    name=nc.get_next_instruction_name(), engine=inst.engine
                )
                nop.sync_info = bass_rust.SyncInfo(on_wait=chunk, on_update=[])
                _orig_add_instruction(self, nop)
            si.on_wait = waits[-MAX_WAITS:]
    _orig_add_instruction(self, inst)


def _patched_drain_and_barrier(self, tick_clock, wait_clock):
    from concourse.tile import ScopedClock

    nc = self.nc
    # Pre-create nops to carry wait overflow (created before the drain so
    # they precede it in program order on the SP engine).
    nops = [nc.sync.nop(nofuse=True) for _ in range(40)]
    drain_inst = nc.sync.drain()
    wait_clock.add_sem_waits(
        drain_inst.ins, ScopedClock({None: tick_clock.global_clock})
    )
    si = drain_inst.ins.sync_info
    waits = list(si.on_wait) if si is not None else []
    if len(waits) > MAX_WAITS:
        chunks = [waits[i : i + MAX_WAITS] for i in range(0, len(waits), MAX_WAITS)]
        assert len(chunks) <= len(nops) + 1, "too many drain waits to split"
        # earlier chunks on the nops, the last chunk stays on the drain
        for nop_inst, chunk in zip(nops, chunks[:-1]):
            nsi = nop_inst.ins.sync_info
            if nsi is None:
                nop_inst.ins.sync_info = bass_rust.SyncInfo(
                    on_wait=chunk, on_update=[]
                )
            else:
                nsi.on_wait = chunk
        si.on_wait = chunks[-1]

    nc.all_engine_barrier()
    assert self.sems is not None
    popped = nc._tile_sem_poison_stack.pop()
    assert popped is self._sem_poison
    nc.clear_and_free_semaphores(list(self.sems.allocated().values()))
    nc.all_engine_barrier()


def install():
    global _installed
    if not _installed:
        tile.TileContext._drain_and_barrier = _patched_drain_and_barrier
        tile.TileContext._add_instruction = _patched_add_instruction
        _installed = True


def split_multi_waits(nc, limit=MAX_WAITS, verbose=False):
    """Safety net: move excess sync-waits onto the nearest preceding
    same-engine instruction with spare wait capacity (waiting earlier is
    always safe in straight-line code)."""
    n_moved = 0
    for f in nc.m.functions:
        for bb in f.blocks:
            insts = bb.instructions
            for i, inst in enumerate(insts):
                si = inst.sync_info
                if si is None:
                    continue
                waits = list(si.on_wait)
                if len(waits) <= limit:
                    continue
                excess = waits[:-limit]
                si.on_wait = waits[-limit:]
                j = i - 1
                while excess and j >= 0:
                    prev = insts[j]
                    if prev.engine == inst.engine:
                        psi = prev.sync_info
                        pw = list(psi.on_wait) if psi is not None else []
                        room = limit - len(pw)
                        if room > 0:
                            take = excess[-room:]
                            excess = excess[:-room]
                            neww = take + pw
                            if psi is None:
                                prev.sync_info = bass_rust.SyncInfo(
                                    on_wait=neww, on_update=[]
                                )
                            else:
                                psi.on_wait = neww
                            n_moved += len(take)
                    j -= 1
                if excess:
                    raise RuntimeError(
                        f"split_multi_waits: no room before {inst.name} in "
                        f"{bb.name} for {len(excess)} waits"
                    )
    if verbose and n_moved:
        print(f"split_multi_waits: moved {n_moved} waits")
    return n_moved

install()
# ---- end inlined workaround ----

FP32 = mybir.dt.float32
BF16 = mybir.dt.bfloat16
AF = mybir.ActivationFunctionType
ALU = mybir.AluOpType
AX = mybir.AxisListType

B, DIM, H, W = 4, 64, 88, 88
L = H * W                  # 7744
D, N, R, K = 32, 64, 2, 4
EPS = 1e-5
INV_CNT = 1.0 / (2 * B * L)
LT = [(i * 512, min(512, L - i * 512)) for i in range((L + 511) // 512)]
CHUNKS = [(0, 2048), (2048, 2048), (4096, 2048), (6144, 1600)]
NT = 16
QL = L // 4                # quarter-L for scratch-limited composites

_cache = {}


def _subtiles(off, ln):
    out = []
    p = 0
    while p < ln:
        w = min(512, ln - p)
        out.append((off + p, p, w))
        p += w
    return out


def build():
    nc = bass.Bass()

    def inp(name, shape):
        return nc.declare_dram_parameter(name, list(shape), FP32, isOutput=False)

    xsrc_d = inp("xsrc", (64, L))
    ysrc_d = inp("ysrc", (64, L))
    toxwT_d = inp("toxwT", (64, 32))
    toxb_d = inp("toxb", (32, 1))
    toywT_d = inp("toywT", (64, 64))
    toyb_d = inp("toyb", (64, 1))
    gx_d = inp("gx", (64, 1))
    bx_d = inp("bxp", (64, 1))
    gy_d = inp("gy", (64, 1))
    by_d = inp("byp", (64, 1))
    mrgb_d = inp("mrgb", (64, 1))
    xyBT_d = inp("xyBT", (32, 2 * 64))
    xyCT_d = inp("xyCT", (32, 2 * 64))
    wdtT_d = inp("wdtT", (32, 2 * 32))
    dtb_d = inp("dtb", (32, 2))
    convdiag_d = inp("convdiag", (32, 2 * 128))
    convb_d = inp("convb", (32, 2))
    dsk_d = inp("dsk", (32, 2))
    wfT_d = inp("wfT", (64, 64))
    projb_d = inp("projb", (64, 1))
    w1T_d = inp("w1T", (64, 4))
    w2T_d = inp("w2T", (4, 64))
    spw_d = inp("spw", (14, 7))
    n3g_d = inp("n3g", (128, 1))
    n3b_d = inp("n3b", (128, 1))
    cbrT_d = inp("cbrT", (128, 9 * 128))
    cbrg_d = inp("cbrg", (128, 1))
    cbrb_d = inp("cbrb", (128, 1))
    outwT_d = inp("outwT", (128, 128))
    outb_d = inp("outb", (128, 1))
    mlow_d = inp("mlow", (64, 1))
    acol_d = inp("acol", (128, 16))
    selB_d = inp("selB", (64, 16 * 128))
    sel32_d = inp("sel32", (128, 32))

    out_d = nc.declare_dram_parameter("out", [128, L], FP32, isOutput=True)

    ar1_in = nc.dram_tensor("ar1_in", [64, 4], FP32)
    ar1_out = nc.dram_tensor("ar1_out", [64, 4], FP32, addr_space="Shared")
    ar2_in = nc.dram_tensor("ar2_in", [64, 4], FP32)
    ar2_out = nc.dram_tensor("ar2_out", [64, 4], FP32, addr_space="Shared")
    ar3_in = nc.dram_tensor("ar3_in", [128, 2], FP32)
    ar3_out = nc.dram_tensor("ar3_out", [128, 2], FP32, addr_space="Shared")
    sp_bounce = nc.dram_tensor("sp_bounce", [1, L], FP32)
    ag_in = nc.dram_tensor("ag_in", [64, L], FP32)
    ag_out = nc.dram_tensor("ag_out", [128, L], FP32)
    GROUPS_ALL = [[0, 1, 2, 3, 4, 5, 6, 7]]
    GROUPS_PAIR = [[0, 4], [1, 5], [2, 6], [3, 7]]

    with tile.TileContext(nc) as tc, \
         tc.tile_pool(name="cols", bufs=1) as cp:

        carries = cp.tile([128, 16], FP32, tag="carries")

        acol = cp.tile([128, 16], FP32, tag="acol")
        nc.sync.dma_start(acol[:], acol_d[:])
        selBf = cp.tile([64, 16 * 128], FP32, tag="selBf")
        nc.sync.dma_start(selBf[:], selB_d[:])
        selK64 = cp.tile([64, 16 * 128], BF16, tag="selK64")
        nc.vector.tensor_copy(selK64[:], selBf[:])
        selK64hi = cp.tile([128, 16 * 128], BF16, tag="selK64hi")
        nc.vector.tensor_copy(selK64hi[64:128, :], selBf[:])
        sel32f = cp.tile([128, 32], FP32, tag="sel32f")
        nc.sync.dma_start(sel32f[:], sel32_d[:])
        sel32 = cp.tile([128, 32], BF16, tag="sel32")
        nc.vector.tensor_copy(sel32[:], sel32f[:])
        epscol = cp.tile([128, 1], FP32, tag="epscol")
        nc.gpsimd.memset(epscol[:], EPS)

        def silu_into(out_ap, in_ap, s1, s2):
            # silu(x) = x * exp(x - ln(1 + exp(x))); fp32 scratches s1, s2
            nc.scalar.activation(s1, in_ap, AF.Exp)
            nc.vector.tensor_scalar_add(s1, s1, 1.0)
            nc.scalar.activation(s1, s1, AF.Ln)
            nc.vector.tensor_sub(s2, in_ap, s1)
            nc.scalar.activation(s2, s2, AF.Exp)
            nc.vector.tensor_mul(out_ap, s2, in_ap)

        def sigmoid_into(out_ap, in_ap, s1):
            # in_ap must differ from out_ap
            nc.scalar.activation(s1, in_ap, AF.Exp)
            nc.vector.tensor_scalar_add(s1, s1, 1.0)
            nc.scalar.activation(s1, s1, AF.Ln)
            nc.vector.tensor_sub(out_ap, in_ap, s1)
            nc.scalar.activation(out_ap, out_ap, AF.Exp)

        def rsqrt_into(out_ap, in_ap, g_ap):
            p = in_ap.partition_size()
            nc.scalar.activation(out_ap, in_ap, AF.Ln, bias=epscol[:p, :])
            nc.scalar.activation(out_ap, out_ap, AF.Exp, scale=-0.5)
            nc.vector.tensor_mul(out_ap, out_ap, g_ap)

        # P_zy rows: 0:64 zyf (z 0:32, yf 32:64) -> vconv@0:32 -> ysF@0:32,
        # ysR@32:64; 64:96 sz; 96:128 x_proj.  Lives P1..P6.
        pmid_cm = tc.tile_pool(name="pmid", bufs=1)
        pmid = pmid_cm.__enter__()
        P_zy = pmid.tile([128, L], FP32, tag="P_zy")
        P_xc = pmid.tile([64, L], BF16, tag="P_xc")

        # scan-phase big operands, freed after P5
        with tc.tile_pool(name="psc", bufs=1) as psc:
            P_dt = psc.tile([64, L], FP32, tag="P_dt")
            P_BCF = psc.tile([128, L], BF16, tag="P_BCF")
            P_BCR = psc.tile([128, L], BF16, tag="P_BCR")

            # ================= P1+P2 =================
            with tc.tile_pool(name="pA", bufs=1) as pA, \
                 tc.tile_pool(name="pAs", bufs=2, space="PSUM") as psA:
                xs = pA.tile([64, L], FP32, tag="xs")
                ysr = pA.tile([64, L], FP32, tag="ysr")
                nc.sync.dma_start(xs[:], xsrc_d[:])
                nc.sync.dma_start(ysr[:], ysrc_d[:])
                scr = P_zy[64:128, :]  # scratch before sz/x_proj written
                st = cp.tile([64, 4], FP32, tag="st")
                nc.scalar.activation(scr, xs[:], AF.Identity,
                                     accum_out=st[:, 0:1])
                nc.scalar.activation(scr, xs[:], AF.Square,
                                     accum_out=st[:, 1:2])
                nc.scalar.activation(scr, ysr[:], AF.Identity,
                                     accum_out=st[:, 2:3])
                nc.scalar.activation(scr, ysr[:], AF.Square,
                                     accum_out=st[:, 3:4])
                mrgb = cp.tile([64, 1], FP32, tag="mrgb")
                nc.sync.dma_start(mrgb[:], mrgb_d[:])
                mdep = cp.tile([64, 1], FP32, tag="mdep")
                nc.vector.tensor_scalar(mdep[:], mrgb[:], -1.0, 1.0,
                                        ALU.mult, ALU.add)
                ar1s = cp.tile([64, 4], FP32, tag="ar1s")
                tmpc = cp.tile([64, 4], FP32, tag="tmpc")
                for j in range(2):
                    nc.vector.tensor_mul(ar1s[:, j:j+1], st[:, j:j+1], mrgb[:])
                    nc.vector.tensor_mul(tmpc[:, 0:1], st[:, 2+j:3+j], mdep[:])
                    nc.vector.tensor_add(ar1s[:, j:j+1], ar1s[:, j:j+1],
                                         tmpc[:, 0:1])
                    nc.vector.tensor_mul(ar1s[:, 2+j:3+j], st[:, j:j+1],
                                         mdep[:])
                    nc.vector.tensor_mul(tmpc[:, 1:2], st[:, 2+j:3+j], mrgb[:])
                    nc.vector.tensor_add(ar1s[:, 2+j:3+j], ar1s[:, 2+j:3+j],
                                         tmpc[:, 1:2])
                nc.sync.dma_start(ar1_in[:], ar1s[:])
                nc.gpsimd.collective_compute(
                    "AllReduce", ALU.add, replica_groups=GROUPS_ALL,
                    ins=[ar1_in[:]], outs=[ar1_out[:]])
                sums = cp.tile([64, 4], FP32, tag="sums")
                nc.sync.dma_start(sums[:], ar1_out[:])
                mv = cp.tile([64, 4], FP32, tag="mv")
                for j, (cs, cq) in enumerate(((0, 1), (2, 3))):
                    nc.vector.tensor_scalar_mul(mv[:, 2*j:2*j+1],
                                                sums[:, cs:cs+1], INV_CNT)
                    nc.vector.tensor_scalar_mul(mv[:, 2*j+1:2*j+2],
                                                sums[:, cq:cq+1], INV_CNT)
                    nc.vector.tensor_mul(tmpc[:, 2:3], mv[:, 2*j:2*j+1],
                                         mv[:, 2*j:2*j+1])
                    nc.vector.tensor_sub(mv[:, 2*j+1:2*j+2],
                                         mv[:, 2*j+1:2*j+2], tmpc[:, 2:3])
                sel = cp.tile([64, 4], FP32, tag="selstats")
                for j in range(2):
                    nc.vector.tensor_mul(sel[:, j:j+1], mv[:, j:j+1], mrgb[:])
                    nc.vector.tensor_mul(tmpc[:, 0:1], mv[:, 2+j:3+j], mdep[:])
                    nc.vector.tensor_add(sel[:, j:j+1], sel[:, j:j+1],
                                         tmpc[:, 0:1])
                    nc.vector.tensor_mul(sel[:, 2+j:3+j], mv[:, j:j+1],
                                         mdep[:])
                    nc.vector.tensor_mul(tmpc[:, 1:2], mv[:, 2+j:3+j],
                                         mrgb[:])
                    nc.vector.tensor_add(sel[:, 2+j:3+j], sel[:, 2+j:3+j],
                                         tmpc[:, 1:2])
                gx = cp.tile([64, 1], FP32, tag="gx")
                nc.sync.dma_start(gx[:], gx_d[:])
                bxp = cp.tile([64, 1], FP32, tag="bxp")
                nc.sync.dma_start(bxp[:], bx_d[:])
                gy = cp.tile([64, 1], FP32, tag="gy")
                nc.sync.dma_start(gy[:], gy_d[:])
                byp = cp.tile([64, 1], FP32, tag="byp")
                nc.sync.dma_start(byp[:], by_d[:])
                sxc = cp.tile([64, 1], FP32, tag="sxc")
                rsqrt_into(sxc[:], sel[:, 1:2], gx[:])
                bxc = cp.tile([64, 1], FP32, tag="bxc")
                nc.vector.tensor_mul(bxc[:], sel[:, 0:1], sxc[:])
                nc.vector.tensor_sub(bxc[:], bxp[:], bxc[:])
                syc = cp.tile([64, 1], FP32, tag="syc")
                rsqrt_into(syc[:], sel[:, 3:4], gy[:])
                byc = cp.tile([64, 1], FP32, tag="byc")
                nc.vector.tensor_mul(byc[:], sel[:, 2:3], syc[:])
                nc.vector.tensor_sub(byc[:], byp[:], byc[:])

                toxwT = cp.tile([64, 32], FP32, tag="toxwT")
                nc.sync.dma_start(toxwT[:], toxwT_d[:])
                toywT = cp.tile([64, 64], FP32, tag="toywT")
                nc.sync.dma_start(toywT[:], toywT_d[:])
                lhsx = cp.tile([64, 32], FP32, tag="lhsx")
                nc.vector.tensor_scalar_mul(lhsx[:], toxwT[:], sxc[:])
                lhsy = cp.tile([64, 64], FP32, tag="lhsy")
                nc.vector.tensor_scalar_mul(lhsy[:], toywT[:], syc[:])
                toxb = cp.tile([32, 1], FP32, tag="toxb")
                nc.sync.dma_start(toxb[:], toxb_d[:])
                toyb = cp.tile([64, 1], FP32, tag="toyb")
                nc.sync.dma_start(toyb[:], toyb_d[:])
                pbias = psA.tile([64, 1], FP32, tag="pbias")
                bxf = cp.tile([32, 1], FP32, tag="bxf")
                nc.tensor.matmul(pbias[:32, :], toxwT[:], bxc[:], start=True,
                                 stop=True)
                nc.scalar.activation(bxf[:], pbias[:32, :], AF.Identity,
                                     bias=toxb[:])
                byf = cp.tile([64, 1], FP32, tag="byf")
                nc.tensor.matmul(pbias[:], toywT[:], byc[:], start=True,
                                 stop=True)
                nc.scalar.activation(byf[:], pbias[:], AF.Identity,
                                     bias=toyb[:])

                # x_proj -> P_zy[96:128]; zyf -> P_zy[32:96]
                pmm = psA.tile([64, 512], FP32, tag="pmm")
                for (l0, w) in LT:
                    nc.tensor.matmul(pmm[:32, :w], lhsx[:], xs[:, l0:l0+w],
                                     start=True, stop=True)
                    nc.scalar.activation(P_zy[96:128, l0:l0+w], pmm[:32, :w],
                                         AF.Identity, bias=bxf[:])
                for (l0, w) in LT:
                    nc.tensor.matmul(pmm[:, :w], lhsy[:], ysr[:, l0:l0+w],
                                     start=True, stop=True)
                    nc.scalar.activation(P_zy[0:64, l0:l0+w], pmm[:, :w],
                                         AF.Identity, bias=byf[:])

                xyBT = cp.tile([32, 2 * 64], FP32, tag="xyBT")
                nc.sync.dma_start(xyBT[:], xyBT_d[:])
                xyCT = cp.tile([32, 2 * 64], FP32, tag="xyCT")
                nc.sync.dma_start(xyCT[:], xyCT_d[:])
                wdtT = cp.tile([32, 2 * 32], FP32, tag="wdtT")
                nc.sync.dma_start(wdtT[:], wdtT_d[:])
                # base-64 copies (matmul needs lhsT/rhs at same base partition)
                xyBTh = cp.tile([64, 2 * 64], FP32, tag="xyBTh")
                nc.sync.dma_start(xyBTh[32:64, :], xyBT_d[:])
                xyCTh = cp.tile([64, 2 * 64], FP32, tag="xyCTh")
                nc.sync.dma_start(xyCTh[32:64, :], xyCT_d[:])
                wdtTh = cp.tile([64, 2 * 32], FP32, tag="wdtTh")
                nc.sync.dma_start(wdtTh[32:64, :], wdtT_d[:])
                byf32 = cp.tile([32, 1], FP32, tag="byf32")
                nc.vector.tensor_copy(byf32[:], byf[32:64, :])
                dtbc = cp.tile([32, 2], FP32, tag="dtbc")
                nc.sync.dma_start(dtbc[:], dtb_d[:])
                bB = cp.tile([64, 2], FP32, tag="bB")
                bC = cp.tile([64, 2], FP32, tag="bC")
                bdt = cp.tile([32, 2], FP32, tag="bdt")
                yf = P_zy[32:64, :]
                for di in range(2):
                    nc.tensor.matmul(pbias[:], xyBT[:, di*64:(di+1)*64],
                                     byf32[:], start=True, stop=True)
                    nc.scalar.copy(bB[:, di:di+1], pbias[:])
                    nc.tensor.matmul(pbias[:], xyCT[:, di*64:(di+1)*64],
                                     byf32[:], start=True, stop=True)
                    nc.scalar.copy(bC[:, di:di+1], pbias[:])
                    nc.tensor.matmul(pbias[:32, :], wdtT[:, di*32:(di+1)*32],
                                     byf32[:], start=True, stop=True)
                    nc.scalar.activation(bdt[:, di:di+1], pbias[:32, :],
                                         AF.Identity, bias=dtbc[:, di:di+1])
                    dstBC = P_BCF if di == 0 else P_BCR
                    dstB = dstBC[0:64, :]
                    dstC = dstBC[64:128, :]
                    # Bm / Cm: write unreversed; R reversed later
                    for (l0, w) in LT:
                        nc.tensor.matmul(pmm[:, :w],
                                         xyBTh[32:64, di*64:(di+1)*64],
                                         yf[:, l0:l0+w], start=True, stop=True)
                        nc.scalar.activation(dstB[:, l0:l0+w], pmm[:, :w],
                                             AF.Identity, bias=bB[:, di:di+1])
                    for (l0, w) in LT:
                        nc.tensor.matmul(pmm[:, :w],
                                         xyCTh[32:64, di*64:(di+1)*64],
                                         yf[:, l0:l0+w], start=True, stop=True)
                        nc.scalar.activation(dstC[:, l0:l0+w], pmm[:, :w],
                                             AF.Identity, bias=bC[:, di:di+1])
                    # dt: softplus = ln(1+exp(lin+b)) per half to bound scratch
                    drow = P_dt[di*32:(di+1)*32, :]
                    for (l0, w) in LT:
                        nc.tensor.matmul(pmm[:32, :w],
                                         wdtTh[32:64, di*32:(di+1)*32],
                                         yf[:, l0:l0+w], start=True, stop=True)
                        nc.scalar.activation(drow[:, l0:l0+w], pmm[:32, :w],
                                             AF.Exp, bias=bdt[:, di:di+1])
                    nc.vector.tensor_scalar_add(drow, drow, 1.0)
                    nc.scalar.activation(drow, drow, AF.Ln)

            # reverse R halves in place via scratch quarters
            with tc.tile_pool(name="prev", bufs=1) as pv:
                rq = pv.tile([64, QL], FP32, tag="rq")
                for q in range(2):
                    a0, b0 = q * QL, L - (q + 1) * QL
                    nc.vector.tensor_copy(rq[0:32, :], P_dt[32:64, a0:a0+QL])
                    nc.vector.tensor_copy(rq[32:64, :], P_dt[32:64, b0:b0+QL])
                    nc.vector.tensor_copy(P_dt[32:64, a0:a0+QL],
                                          rq[32:64, ::-1])
                    nc.vector.tensor_copy(P_dt[32:64, b0:b0+QL],
                                          rq[0:32, ::-1])
                rq16 = pv.tile([128, QL], BF16, tag="rq16")
                for q in range(2):
                    a0, b0 = q * QL, L - (q + 1) * QL
                    nc.vector.tensor_copy(rq16[:, :], P_BCR[:, a0:a0+QL])
                    nc.vector.tensor_copy(P_BCR[:, a0:a0+QL],
                                          P_BCR[:, b0:b0+QL][:, ::-1])
                    nc.vector.tensor_copy(P_BCR[:, b0:b0+QL], rq16[:, ::-1])

            # ============ P3: sz, conv1d+silu ============
            with tc.tile_pool(name="pB", bufs=1) as pB, \
                 tc.tile_pool(name="pBs", bufs=2, space="PSUM") as psB:
                s1 = pB.tile([32, QL], FP32, tag="s1")
                s2 = pB.tile([32, QL], FP32, tag="s2")
                # sz = silu(z): z in P_zy[0:32]; out P_zy[64:96]
                for q in range(4):
                    a0 = q * QL
                    silu_into(P_zy[64:96, a0:a0+QL], P_zy[0:32, a0:a0+QL],
                              s1[:], s2[:])
                convdiag = cp.tile([32, 2 * 128], FP32, tag="convdiag")
                nc.sync.dma_start(convdiag[:], convdiag_d[:])
                convdiag16 = cp.tile([32, 2 * 128], BF16, tag="convdiag16")
                nc.vector.tensor_copy(convdiag16[:], convdiag[:])
                convb = cp.tile([32, 2], FP32, tag="convb")
                nc.sync.dma_start(convb[:], convb_d[:])
                xpad = pB.tile([32, L + 3], BF16, tag="xpad")
                pc = psB.tile([32, 512], FP32, tag="pc")
                for di in range(2):
                    nc.gpsimd.memset(xpad[:, 0:3], 0.0)
                    if di == 0:
                        nc.vector.tensor_copy(xpad[:, 3:3+L], P_zy[96:128, :])
                    else:
                        nc.vector.tensor_copy(xpad[:, 3:3+L],
                                              P_zy[96:128, ::-1])
                    # conv into vconv = P_zy[0:32] (z dead after sz)
                    vconv = P_zy[0:32, :]
                    for (l0, w) in LT:
                        for k in range(4):
                            nc.tensor.matmul(
                                pc[:, :w],
                                convdiag16[:, di*128+k*32:di*128+(k+1)*32],
                                xpad[:, l0+k:l0+k+w],
                                start=(k == 0), stop=(k == 3))
                        nc.scalar.activation(vconv[:, l0:l0+w], pc[:, :w],
                                             AF.Identity,
                                             bias=convb[:, di:di+1])
                    for q in range(4):
                        a0 = q * QL
                        silu_into(P_xc[di*32:(di+1)*32, a0:a0+QL],
                                  vconv[:, a0:a0+QL], s1[:], s2[:])

            # ================= P5: scans =================
            with tc.tile_pool(name="scp", bufs=2) as sp_, \
                 tc.tile_pool(name="repp", bufs=1) as rp_, \
                 tc.tile_pool(name="scps", bufs=2, space="PSUM") as reps, \
                 tc.tile_pool(name="ysps", bufs=4, space="PSUM") as ysps:
                for di in range(2):
                    BC = P_BCF if di == 0 else P_BCR
                    for (c0, cw) in CHUNKS:
                        dt_rep = rp_.tile([128, 2048], FP32, tag="dt_rep")
                        dtxc_rep = rp_.tile([128, 2048], BF16, tag="dtxc_rep")
                        nc.vector.tensor_mul(
                            dtxc_rep[0:32, :cw], P_dt[di*32:(di+1)*32, c0:c0+cw],
                            P_xc[di*32:(di+1)*32, c0:c0+cw])
                        for q in range(1, 4):
                            nc.gpsimd.tensor_copy(dtxc_rep[32*q:32*(q+1), :cw],
                                                  dtxc_rep[0:32, :cw])
                            nc.gpsimd.tensor_copy(
                                dt_rep[32*q:32*(q+1), :cw],
                                P_dt[di*32:(di+1)*32, c0:c0+cw])
                        nc.gpsimd.tensor_copy(dt_rep[0:32, :cw],
                                              P_dt[di*32:(di+1)*32, c0:c0+cw])
                        ys_subs = []
                        for (a0, p0, w) in _subtiles(c0, cw):
                            yst = ysps.tile([32, 512], FP32, tag="ys")
                            ys_subs.append((yst, a0, p0, w))
                        halves = [(0, min(1024, cw))]
                        if cw > 1024:
                            halves.append((1024, cw - 1024))
                        for t in range(NT):
                            dA = sp_.tile([128, 2048], FP32, tag="dA")
                            nc.scalar.activation(dA[:, :cw], dt_rep[:, :cw],
                                                 AF.Exp, scale=acol[:, t:t+1])
                            lsl = selK64[:, t * 128:(t + 1) * 128]
                            lslC = selK64hi[64:128, t * 128:(t + 1) * 128]
                            dBu = sp_.tile([128, 2048], BF16, tag="dBu")
                            # half-sized B_rep psum tiles (bufs=2) so PE can
                            # run a half ahead of the DVE multiply chain
                            for (hf, hw) in halves:
                                rep = reps.tile([128, 1024], FP32, tag="rep")
                                for (a0, p0, w) in _subtiles(c0 + hf, hw):
                                    nc.tensor.matmul(rep[:, p0:p0+w], lsl,
                                                     BC[0:64, a0:a0+w],
                                                     start=True, stop=True)
                                nc.vector.tensor_mul(dBu[:, hf:hf+hw],
                                                     rep[:, :hw],
                                                     dtxc_rep[:, hf:hf+hw])
                            hs = sp_.tile([128, 2048], FP32, tag="hs")
                            init = 0.0 if c0 == 0 else carries[:, t:t+1]
                            nc.vector.tensor_tensor_scan(
                                hs[:, :cw], dA[:, :cw], dBu[:, :cw], init,
                                ALU.mult, ALU.add)
                            nc.gpsimd.tensor_copy(carries[:, t:t+1],
                                                   hs[:, cw-1:cw])
                            hc = sp_.tile([128, 2048], BF16, tag="hc")
                            for (hf, hw) in halves:
                                rep2 = reps.tile([128, 1024], FP32, tag="rep")
                                for (a0, p0, w) in _subtiles(c0 + hf, hw):
                                    nc.tensor.matmul(rep2[:, p0:p0+w], lslC,
                                                     BC[64:128, a0:a0+w],
                                                     start=True, stop=True)
                                nc.vector.tensor_mul(hc[:, hf:hf+hw],
                                                     hs[:, hf:hf+hw],
                                                     rep2[:, :hw])
                            for (yst, a0, p0, w) in ys_subs:
                                nc.tensor.matmul(yst[:, :w], sel32,
                                                 hc[:, p0:p0+w],
                                                 start=(t == 0),
                                                 stop=(t == NT - 1))
                        for (yst, a0, p0, w) in ys_subs:
                            nc.scalar.copy(
                                P_zy[di*32:32 + di*32, a0:a0+w], yst[:, :w])

        # ============ P6: combine + proj + resid + CBAM ============
        with tc.tile_pool(name="p6", bufs=1) as p6, \
             tc.tile_pool(name="p6small", bufs=2) as p6s, \
             tc.tile_pool(name="p6ps", bufs=2, space="PSUM") as ps6:
            dsk = cp.tile([32, 2], FP32, tag="dsk")
            nc.sync.dma_start(dsk[:], dsk_d[:])
            dsk32b = cp.tile([64, 2], FP32, tag="dsk32b")
            nc.sync.dma_start(dsk32b[32:64, :], dsk_d[:])
            outFR = p6.tile([64, L], FP32, tag="outFR")
            # ys += Dsk*xc; out = ys*silu(z)  (R still in reversed time)
            # (2-input DVE ops need equal input base partitions -> use a
            #  (64,512) staging tile and operate at matching row offsets)
            for di in range(2):
                r0 = di * 32
                yrow = P_zy[r0:r0+32, :]
                for (l0, w) in LT:
                    tmp = p6s.tile([64, 512], FP32, tag="tmp64")
                    nc.vector.tensor_scalar_mul(
                        tmp[r0:r0+32, :w], P_xc[r0:r0+32, l0:l0+w],
                        dsk[:32, di:di+1] if di == 0 else dsk32b[32:64, di:di+1])
                    nc.vector.tensor_add(yrow[:, l0:l0+w], yrow[:, l0:l0+w],
                                         tmp[r0:r0+32, :w])
            for (l0, w) in LT:
                tmp = p6s.tile([64, 512], FP32, tag="tmp64")
                nc.vector.tensor_copy(tmp[0:32, :w], P_zy[64:96, l0:l0+w])
                nc.vector.tensor_mul(outFR[0:32, l0:l0+w],
                                     P_zy[0:32, l0:l0+w], tmp[0:32, :w])
            # outR: multiply reversed-time ysR by reversed sz, then unreverse
            for (l0, w) in LT:
                tmp = p6s.tile([64, 512], FP32, tag="tmp64")
                nc.vector.tensor_copy(tmp[32:64, :w],
                                      P_zy[64:96, L-l0-w:L-l0][:, ::-1])
                nc.vector.tensor_mul(tmp[32:64, :w], P_zy[32:64, l0:l0+w],
                                     tmp[32:64, :w])
                nc.vector.tensor_copy(outFR[32:64, L-l0-w:L-l0],
                                      tmp[32:64, :w][:, ::-1])
            wfT = cp.tile([64, 64], FP32, tag="wfT")
            nc.sync.dma_start(wfT[:], wfT_d[:])
            projb = cp.tile([64, 1], FP32, tag="projb")
            nc.sync.dma_start(projb[:], projb_d[:])
            x2 = p6.tile([64, L], FP32, tag="x2")
            pm6 = ps6.tile([64, 512], FP32, tag="pm6")
            for (l0, w) in LT:
                nc.tensor.matmul(pm6[:, :w], wfT[:], outFR[:, l0:l0+w],
                                 start=True, stop=True)
                nc.scalar.activation(x2[:, l0:l0+w], pm6[:, :w], AF.Identity,
                                     bias=projb[:])
            for (l0, w) in LT:
                rt = p6s.tile([64, 512], FP32, tag="rt")
                nc.sync.dma_start(rt[:, :w], xsrc_d[:, l0:l0+w])
                nc.vector.tensor_add(x2[:, l0:l0+w], x2[:, l0:l0+w],
                                     rt[:, :w])

            # channel attention
            colA = cp.tile([64, 1], FP32, tag="colA")
            colB = cp.tile([64, 1], FP32, tag="colB")
            nc.vector.tensor_reduce(colA[:], x2[:], AX.X, ALU.add)
            nc.vector.tensor_scalar_mul(colA[:], colA[:], 1.0 / L)
            nc.vector.tensor_reduce(colB[:], x2[:], AX.X, ALU.max)
            w1T = cp.tile([64, 4], FP32, tag="w1T")
            nc.sync.dma_start(w1T[:], w1T_d[:])
            w2T = cp.tile([4, 64], FP32, tag="w2T")
            nc.sync.dma_start(w2T[:], w2T_d[:])
            pml = ps6.tile([4, 1], FP32, tag="small")
            rl = cp.tile([4, 2], FP32, tag="rl")
            nc.tensor.matmul(pml[:], w1T[:], colA[:], start=True, stop=True)
            nc.scalar.activation(rl[:, 0:1], pml[:], AF.Relu)
            nc.tensor.matmul(pml[:], w1T[:], colB[:], start=True, stop=True)
            nc.scalar.activation(rl[:, 1:2], pml[:], AF.Relu)
            pca = ps6.tile([64, 1], FP32, tag="small")
            nc.tensor.matmul(pca[:], w2T[:], rl[:, 0:1], start=True,
                             stop=False)
            nc.tensor.matmul(pca[:], w2T[:], rl[:, 1:2], start=False,
                             stop=True)
            cac = cp.tile([64, 1], FP32, tag="cac")
            ct0 = cp.tile([64, 1], FP32, tag="ct0")
            ct1 = cp.tile([64, 1], FP32, tag="ct1")
            nc.scalar.copy(ct0[:], pca[:])
            sigmoid_into(cac[:], ct0[:], ct1[:])
            nc.vector.tensor_scalar_mul(x2[:], x2[:], cac[:])

            # spatial attention: stack rows (c,i), layout (88, 94) per row
            SW = 94
            stack = p6.tile([14, H * SW], BF16, tag="stack")
            nc.gpsimd.memset(stack[:], 0.0)
            ones64 = cp.tile([64, 1], FP32, tag="ones64")
            nc.gpsimd.memset(ones64[:], 1.0)
            HG = [(h0, min(5, H - h0)) for h0 in range(0, H, 5)]
            st3d = stack[:, :].rearrange("p (h w) -> p h w", w=SW)
            pg = ps6.tile([1, 512], FP32, tag="small")
            # mean row -> stack row 3 ; max row -> stack row 10
            # (engine outputs must sit at base partition 0; DMA into rows)
            for (h0, hc_) in HG:
                nc.tensor.matmul(pg[:, :hc_*W], ones64[:],
                                 x2[:, h0*W:(h0+hc_)*W], start=True, stop=True)
                srm = p6s.tile([1, 512], FP32, tag="srm")
                nc.scalar.activation(srm[:, :hc_*W], pg[:, :hc_*W], AF.Copy,
                                     scale=1.0 / 64)
                nc.gpsimd.dma_start(
                    st3d[3:4, h0:h0+hc_, 3:3+W],
                    srm[:, :hc_*W].rearrange("p (h w) -> p h w", w=W))
                srx = p6s.tile([1, 512], FP32, tag="srx")
                nc.gpsimd.tensor_reduce(
                    srx[:, :hc_*W],
                    x2[:, h0*W:(h0+hc_)*W], AX.C, ALU.max)
                nc.gpsimd.dma_start(
                    st3d[10:11, h0:h0+hc_, 3:3+W],
                    srx[:, :hc_*W].rearrange("p (h w) -> p h w", w=W))
            # shifted copies for i != 3
            for c_ in range(2):
                src_r = 3 if c_ == 0 else 10
                for i_ in range(7):
                    r = c_ * 7 + i_
                    if i_ == 3:
                        continue
                    sh = i_ - 3
                    h_lo = max(0, -sh)
                    h_hi = min(H, H - sh)
                    nc.sync.dma_start(
                        st3d[r:r+1, h_lo:h_hi, 3:3+W],
                        st3d[src_r:src_r+1, h_lo+sh:h_hi+sh, 3:3+W])
            spw = cp.tile([14, 7], FP32, tag="spw")
            nc.sync.dma_start(spw[:], spw_d[:])
            spw16 = cp.tile([14, 7], BF16, tag="spw16")
            nc.vector.tensor_copy(spw16[:], spw[:])
            psa = ps6.tile([1, 512], FP32, tag="small")
            srow = p6s.tile([1, 512], FP32, tag="srow")
            for (h0, hc_) in HG:
                for jj in range(7):
                    rhs = st3d[:, h0:h0+hc_, jj:jj+W]
                    nc.tensor.matmul(psa[:, :hc_*W], spw16[:, jj:jj+1], rhs,
                                     start=(jj == 0), stop=(jj == 6))
                srow = p6s.tile([1, 512], FP32, tag="srow")
                nc.scalar.copy(srow[:, :hc_*W], psa[:, :hc_*W])
                nc.sync.dma_start(sp_bounce[0:1, h0*W:(h0+hc_)*W],
                                  srow[:, :hc_*W])
            sa2d = p6s.tile([88, 88], FP32, tag="sa2d")
            nc.sync.dma_start(sa2d[:],
                              sp_bounce[0, :].rearrange("(h w) -> h w", w=W))
            s2a = p6s.tile([88, 88], FP32, tag="s2a")
            s2c = p6s.tile([88, 88], FP32, tag="s2c")
            sigmoid_into(s2c[:], sa2d[:], s2a[:])
            nc.sync.dma_start(sp_bounce[0, :].rearrange("(h w) -> h w", w=W),
                              s2c[:])
            ones1 = cp.tile([1, 64], FP32, tag="ones1")
            nc.gpsimd.memset(ones1[:], 1.0)
            for (l0, w) in LT:
                sarow = p6s.tile([1, 512], FP32, tag="sarow")
                nc.sync.dma_start(sarow[:, :w], sp_bounce[:, l0:l0+w])
                pbc = ps6.tile([64, 512], FP32, tag="pm6")
                nc.tensor.matmul(pbc[:, :w], ones1[:], sarow[:, :w],
                                 start=True, stop=True)
                nc.vector.tensor_mul(outFR[:, l0:l0+w], x2[:, l0:l0+w],
                                     pbc[:, :w])
            x3 = outFR  # reuse

            nc.sync.dma_start(ag_in[:], x3[:])
            st2 = cp.tile([64, 2], FP32, tag="st2")
            nc.scalar.activation(x2[:], x3[:], AF.Identity,
                                 accum_out=st2[:, 0:1])
            nc.scalar.activation(x2[:], x3[:], AF.Square,
                                 accum_out=st2[:, 1:2])
            mlow = cp.tile([64, 1], FP32, tag="mlow")
            nc.sync.dma_start(mlow[:], mlow_d[:])
            mhigh = cp.tile([64, 1], FP32, tag="mhigh")
            nc.vector.tensor_scalar(mhigh[:], mlow[:], -1.0, 1.0, ALU.mult,
                                    ALU.add)
            ar2s = cp.tile([64, 4], FP32, tag="ar2s")
            for j in range(2):
                nc.vector.tensor_mul(ar2s[:, j:j+1], st2[:, j:j+1], mlow[:])
                nc.vector.tensor_mul(ar2s[:, 2+j:3+j], st2[:, j:j+1],
                                     mhigh[:])
            nc.sync.dma_start(ar2_in[:], ar2s[:])
            nc.gpsimd.collective_compute(
                "AllReduce", ALU.add, replica_groups=GROUPS_ALL,
                ins=[ar2_in[:]], outs=[ar2_out[:]])
            nc.gpsimd.collective_compute(
                "AllGather", ALU.bypass, replica_groups=GROUPS_PAIR,
                ins=[ag_in[:]], outs=[ag_out[:]])

        pmid_cm.__exit__(None, None, None)

        # ============ P9: FFN ============
        with tc.tile_pool(name="p9", bufs=1) as p9, \
             tc.tile_pool(name="p9s", bufs=2, space="PSUM") as ps9:
            rgbd = p9.tile([128, L], FP32, tag="rgbd")
            nc.sync.dma_start(rgbd[:], ag_out[:])
            s2st = cp.tile([64, 4], FP32, tag="s2st")
            nc.sync.dma_start(s2st[:], ar2_out[:])
            n3g = cp.tile([128, 1], FP32, tag="n3g")
            nc.sync.dma_start(n3g[:], n3g_d[:])
            n3b = cp.tile([128, 1], FP32, tag="n3b")
            nc.sync.dma_start(n3b[:], n3b_d[:])
            sc128 = cp.tile([128, 1], FP32, tag="sc128")
            sh128 = cp.tile([128, 1], FP32, tag="sh128")
            mvt = cp.tile([64, 2], FP32, tag="mvt")
            ctA = cp.tile([64, 1], FP32, tag="ctA")
            gh = cp.tile([64, 1], FP32, tag="gh")
            bh = cp.tile([64, 1], FP32, tag="bh")
            sch = cp.tile([64, 1], FP32, tag="sch")
            shh = cp.tile([64, 1], FP32, tag="shh")
            for half in range(2):
                r0 = half * 64
                nc.vector.tensor_copy(gh[:], n3g[r0:r0+64, :])
                nc.vector.tensor_copy(bh[:], n3b[r0:r0+64, :])
                nc.vector.tensor_scalar_mul(mvt[:, 0:1],
                                            s2st[:, 2*half:2*half+1], INV_CNT)
                nc.vector.tensor_scalar_mul(mvt[:, 1:2],
                                            s2st[:, 2*half+1:2*half+2],
                                            INV_CNT)
                nc.vector.tensor_mul(ctA[:], mvt[:, 0:1], mvt[:, 0:1])
                nc.vector.tensor_sub(mvt[:, 1:2], mvt[:, 1:2], ctA[:])
                rsqrt_into(sch[:], mvt[:, 1:2], gh[:])
                nc.vector.tensor_mul(ctA[:], mvt[:, 0:1], sch[:])
                nc.vector.tensor_sub(shh[:], bh[:], ctA[:])
                nc.vector.tensor_copy(sc128[r0:r0+64, :], sch[:])
                nc.vector.tensor_copy(sh128[r0:r0+64, :], shh[:])
            n316 = p9.tile([128, L], BF16, tag="n316")
            nc.vector.tensor_scalar(n316[:], rgbd[:], sc128[:], sh128[:],
                                    ALU.mult, ALU.add)
            PW = 90
            n3pad = p9.tile([128, 90 * PW], BF16, tag="n3pad")
            nc.gpsimd.memset(n3pad[:], 0.0)
            nc.sync.dma_start(
                n3pad[:, :].rearrange("p (h w) -> p h w", w=PW)[:, 1:89, 1:89],
                n316[:, :].rearrange("p (h w) -> p h w", w=W))
            cbrT16 = p9.tile([128, 9 * 128], BF16, tag="cbrT16")
            with tc.tile_pool(name="pcl", bufs=1) as pcl:
                cbrT = pcl.tile([128, 9 * 128], FP32, tag="cbrT")
                nc.sync.dma_start(cbrT[:], cbrT_d[:])
                nc.vector.tensor_copy(cbrT16[:], cbrT[:])
            h3 = p9.tile([128, L], FP32, tag="h3")
            pc9 = ps9.tile([128, 440], FP32, tag="pc9")
            HG = [(h0, min(5, H - h0)) for h0 in range(0, H, 5)]
            n3p3 = n3pad[:, :].rearrange("p (h w) -> p h w", w=PW)
            for (h0, hc_) in HG:
                for ij in range(9):
                    i_, j_ = ij // 3, ij % 3
                    rhs = n3p3[:, h0+i_:h0+i_+hc_, j_:j_+W]
                    nc.tensor.matmul(pc9[:, :hc_*W],
                                     cbrT16[:, ij*128:(ij+1)*128], rhs,
                                     start=(ij == 0), stop=(ij == 8))
                nc.scalar.copy(h3[:, h0*W:(h0+hc_)*W], pc9[:, :hc_*W])
            st3 = cp.tile([128, 2], FP32, tag="st3")
            hr = p9.tile([128, L], FP32, tag="hr")
            nc.scalar.activation(hr[:], h3[:], AF.Identity,
                                 accum_out=st3[:, 0:1])
            nc.scalar.activation(hr[:], h3[:], AF.Square,
                                 accum_out=st3[:, 1:2])
            nc.sync.dma_start(ar3_in[:], st3[:])
            nc.gpsimd.collective_compute(
                "AllReduce", ALU.add, replica_groups=GROUPS_ALL,
                ins=[ar3_in[:]], outs=[ar3_out[:]])
            st3o = cp.tile([128, 2], FP32, tag="st3o")
            nc.sync.dma_start(st3o[:], ar3_out[:])
            cbrg = cp.tile([128, 1], FP32, tag="cbrg")
            nc.sync.dma_start(cbrg[:], cbrg_d[:])
            cbrb = cp.tile([128, 1], FP32, tag="cbrb")
            nc.sync.dma_start(cbrb[:], cbrb_d[:])
            m3c = cp.tile([128, 1], FP32, tag="m3c")
            v3c = cp.tile([128, 1], FP32, tag="v3c")
            ct3 = cp.tile([128, 1], FP32, tag="ct3")
            nc.vector.tensor_scalar_mul(m3c[:], st3o[:, 0:1], INV_CNT)
            nc.vector.tensor_scalar_mul(v3c[:], st3o[:, 1:2], INV_CNT)
            nc.vector.tensor_mul(ct3[:], m3c[:], m3c[:])
            nc.vector.tensor_sub(v3c[:], v3c[:], ct3[:])
            sc3 = cp.tile([128, 1], FP32, tag="sc3")
            rsqrt_into(sc3[:], v3c[:], cbrg[:])
            sh3 = cp.tile([128, 1], FP32, tag="sh3")
            nc.vector.tensor_mul(sh3[:], m3c[:], sc3[:])
            nc.vector.tensor_sub(sh3[:], cbrb[:], sh3[:])
            nc.vector.tensor_scalar(hr[:], h3[:], sc3[:], sh3[:], ALU.mult,
                                    ALU.add)
            nc.scalar.activation(hr[:], hr[:], AF.Relu)
            outwT = cp.tile([128, 128], FP32, tag="outwT")
            nc.sync.dma_start(outwT[:], outwT_d[:])
            outb = cp.tile([128, 1], FP32, tag="outb")
            nc.sync.dma_start(outb[:], outb_d[:])
            fin = h3  # reuse
            pf = ps9.tile([128, 512], FP32, tag="pf")
            for (l0, w) in LT:
                nc.tensor.matmul(pf[:, :w], outwT[:], hr[:, l0:l0+w],
                                 start=True, stop=True)
                nc.scalar.activation(fin[:, l0:l0+w], pf[:, :w], AF.Identity,
                                     bias=outb[:])
            nc.vector.tensor_add(fin[:], fin[:], rgbd[:])
            nc.sync.dma_start(out_d[:], fin[:])

    return nc


def _host_prep(rgb, depth, params):
    rgbf = np.ascontiguousarray(np.asarray(rgb, np.float32).reshape(B, DIM, L))
    depf = np.ascontiguousarray(np.asarray(depth, np.float32).reshape(B, DIM, L))

    def n32(x):
        return np.ascontiguousarray(np.asarray(x, np.float32))

    acol = np.zeros((128, 16), np.float32)
    for t in range(16):
        for row in range(128):
            acol[row, t] = -(4 * t + row // 32 + 1)
    selB = np.zeros((64, 16 * 128), np.float32)
    for t in range(16):
        for m in range(128):
            selB[4 * t + m // 32, t * 128 + m] = 1.0
    sel32 = np.zeros((128, 32), np.float32)
    for row in range(128):
        sel32[row, row % 32] = 1.0

    in_maps = []
    for c in range(8):
        blk, b = c // 4, c % 4
        p = params['d2r'] if blk == 0 else params['r2d']
        pcb = params['cbam1'] if blk == 0 else params['cbam2']
        if blk == 0:
            xs, ys_ = rgbf[b], depf[b]
            gx, bx = n32(params['rgb_g']), n32(params['rgb_b'])
            gy, by = n32(params['dep_g']), n32(params['dep_b'])
            mrgb = 1.0
        else:
            xs, ys_ = depf[b], rgbf[b]
            gx, bx = n32(params['dep_g']), n32(params['dep_b'])
            gy, by = n32(params['rgb_g']), n32(params['rgb_b'])
            mrgb = 0.0
        mF, mR = p['mF'], p['mR']
        xyBT = np.concatenate(
            [n32(m['xy_w'])[R:R+N].T for m in (mF, mR)], 1)
        xyCT = np.concatenate(
            [n32(m['xy_w'])[R+N:].T for m in (mF, mR)], 1)
        wdtT = np.concatenate(
            [(n32(m['dt_w']) @ n32(m['xy_w'])[:R]).T for m in (mF, mR)], 1)
        dtb = np.stack([n32(m['dt_b']) for m in (mF, mR)], 1)
        convdiag = np.concatenate(
            [np.concatenate([np.diag(n32(m['conv_w'])[:, k]) for k in range(4)],
                            1) for m in (mF, mR)], 1)
        convb = np.stack([n32(m['conv_b']) for m in (mF, mR)], 1)
        dsk = np.stack([n32(m['Dsk']) for m in (mF, mR)], 1)
        bd = np.zeros((64, 64), np.float32)
        bd[:32, :32] = n32(mF['out_w'])
        bd[32:, 32:] = n32(mR['out_w'])
        wf = n32(p['proj_w']) @ bd
        ffn = params['ffn']
        cbrT = np.concatenate(
            [n32(ffn['cbr_w'])[:, :, ij // 3, ij % 3].T for ij in range(9)], 1)
        spw = n32(pcb['sp_w'])[0].reshape(14, 7)
        in_maps.append({
            "xsrc": xs, "ysrc": ys_,
            "toxwT": n32(p['to_x_w']).T.copy(), "toxb": n32(p['to_x_b'])[:, None],
            "toywT": n32(p['to_y_w']).T.copy(), "toyb": n32(p['to_y_b'])[:, None],
            "gx": gx[:, None], "bxp": bx[:, None],
            "gy": gy[:, None], "byp": by[:, None],
            "mrgb": np.full((64, 1), mrgb, np.float32),
            "xyBT": np.ascontiguousarray(xyBT),
            "xyCT": np.ascontiguousarray(xyCT),
            "wdtT": np.ascontiguousarray(wdtT),
            "dtb": np.ascontiguousarray(dtb),
            "convdiag": np.ascontiguousarray(convdiag),
            "convb": np.ascontiguousarray(convb),
            "dsk": np.ascontiguousarray(dsk),
            "wfT": np.ascontiguousarray(wf.T.copy()),
            "projb": n32(p['proj_b'])[:, None],
            "w1T": n32(pcb['w1']).T.copy(), "w2T": n32(pcb['w2']).T.copy(),
            "spw": np.ascontiguousarray(spw),
            "n3g": n32(params['n3_g'])[:, None],
            "n3b": n32(params['n3_b'])[:, None],
            "cbrT": np.ascontiguousarray(cbrT),
            "cbrg": n32(ffn['cbr_g'])[:, None],
            "cbrb": n32(ffn['cbr_b'])[:, None],
            "outwT": n32(ffn['out_w']).T.copy(),
            "outb": n32(ffn['out_b'])[:, None],
            "mlow": np.full((64, 1), 1.0 if blk == 0 else 0.0, np.float32),
            "acol": acol, "selB": selB, "sel32": sel32,
        })
    return in_maps


def kernel(rgb, depth, params):
    if "nc" not in _cache:
        _cache["nc"] = build()
    nc = _cache["nc"]
    in_maps = _host_prep(rgb, depth, params)
    res = bass_utils.run_bass_kernel_spmd(nc, in_maps, list(range(8)))
    out = np.stack([res.results[b]["out"].reshape(128, H, W)
                    for b in range(B)])
    return out.astype(np.float32)


if __name__ == "__main__":
    import jax
    jax.config.update('jax_platforms', 'cpu')
    import reference as ref
    inputs = ref.setup_inputs()
    expected = np.asarray(ref.reference(**inputs))
    got = kernel(np.asarray(inputs['rgb']), np.asarray(inputs['depth']),
                 inputs['params'])
    err = np.abs(got - expected)
    den = np.abs(expected).mean()
    print("max abs err:", err.max(), "mean rel:", err.mean() / den)
